# revision 17
# baseline (speedup 1.0000x reference)
"""Trainium2 Bass kernel: GPT-style causal self-attention block.

Computes, for x[B=4, T=2048, C=1024], 16 heads x 64 dims:
    qkv = x @ w_attn + b_attn ; causal softmax attention ; y @ w_proj + b_proj

Sharding (8 cores): data-parallel over B (4) x tensor-parallel over head
groups (2 groups of 8 heads, Megatron style).  Each core:
  - receives x^T (host-transposed) and its slice of the weights,
  - computes Q^T/K^T (head-pair packed on partitions) and token-major V,
  - runs causal attention per head-pair: the two heads' S^T matmuls sit on
    disjoint PE row groups (partitions 0-63 / 64-127) so they execute
    concurrently on the 128x128 array; one ScalarE exp instruction covers
    both heads' tiles; AV matmuls carry a ones-column so the softmax
    denominators fall out of the same accumulation,
  - normalization is deferred off the PSUM critical path (single DVE copy
    evacuates y+sums, then recip/broadcast/scale from SBUF),
  - applies its row-slice of w_proj (row-parallel) producing a partial
    [T, C] output.  Host sums the two partials per batch and adds b_proj.

The per-512-token-segment loop interleaves QKV -> attention -> proj so the
TensorE-heavy projection work overlaps the ScalarE-heavy softmax work.
"""

import os
import ml_dtypes
import numpy as np

B, T, C = 4, 2048, 1024
N_HEAD = 16
D = 64  # head dim
H_LOC = 8  # heads per core
N_CORES = 8

NTB = T // 128   # 16 token blocks
NCB = C // 128   # 8 contraction blocks
NSEG = T // 512  # 4 token segments
QQ = 512         # attention q-tile width

_cache = {}
_dbg_tensors = {}

last_exec_ns = None


def _build_program(reps=1, phases='ABCD', opts=()):
    from contextlib import ExitStack

    import concourse.bass as bass
    import concourse.mybir as mybir
    import concourse.tile as tile
    from concourse import bacc

    f32 = mybir.dt.float32
    bf16 = mybir.dt.bfloat16
    AF = mybir.ActivationFunctionType

    import concourse.hw_specs as hw_specs
    _patch = {}
    if 'pe32' in opts:
        # scheduling-only hint: measured HW bf16 matmul throughput is
        # ~3.24 G cols/s in steady state (mm512x8 microbench: 158 ns per
        # 512-col K=128 matmul); restored before return
        _patch = {"PE_CYCLE": hw_specs.TRN2Spec.PE_CYCLE,
                  "PE_CYCLE_PSTATE_MID": hw_specs.TRN2Spec.PE_CYCLE_PSTATE_MID}
        hw_specs.TRN2Spec.PE_CYCLE = 1e9 / 3.24e9
        hw_specs.TRN2Spec.PE_CYCLE_PSTATE_MID = 1e9 / 3.24e9
    elif 'fastpe' in opts:
        # scheduling-only hint: match the cost model to measured HW matmul
        # throughput (bf16 ~4x the default model) while building; restored
        # before return so no global state leaks
        _patch = {"PE_CYCLE": hw_specs.TRN2Spec.PE_CYCLE,
                  "PE_CYCLE_PSTATE_MID": hw_specs.TRN2Spec.PE_CYCLE_PSTATE_MID}
        hw_specs.TRN2Spec.PE_CYCLE = 1e9 / 9.6e9
        hw_specs.TRN2Spec.PE_CYCLE_PSTATE_MID = 1e9 / 4.8e9

    nc = bacc.Bacc("TRN2", target_bir_lowering=False, debug=False,
                   num_devices=N_CORES)

    xt_d = nc.dram_tensor("xt", [C, T], bf16, kind="ExternalInput")
    wqkv_d = nc.dram_tensor("wqkv", [C, 1536], bf16, kind="ExternalInput")
    bqkv_d = nc.dram_tensor("bqkv", [1536], f32, kind="ExternalInput")
    wp_d = nc.dram_tensor("wproj", [512, C], bf16, kind="ExternalInput")
    out_d = nc.dram_tensor("out", [T, C], bf16, kind="ExternalOutput")
    dbg = 'dbg' in opts
    if dbg:
        qkT_d = nc.dram_tensor("qkT_dbg", [128, 8, T], bf16,
                               kind="ExternalOutput")
        v_d = nc.dram_tensor("v_dbg", [128, H_LOC, NTB, 65], bf16,
                             kind="ExternalOutput")
        yt_d = nc.dram_tensor("yt_dbg", [128, 4, T], bf16,
                              kind="ExternalOutput")
        sp_d = nc.dram_tensor("sp_dbg", [128, 2, 512], f32,
                              kind="ExternalOutput")
        pt_d = nc.dram_tensor("pt_dbg", [128, 2, 512], bf16,
                              kind="ExternalOutput")
        ytm_d = nc.dram_tensor("ytm_dbg", [65, 512], f32,
                               kind="ExternalOutput")
        recip_d = nc.dram_tensor("recip_dbg", [1, 512], f32,
                                 kind="ExternalOutput")
        bc_d = nc.dram_tensor("bc_dbg", [64, 512], f32,
                              kind="ExternalOutput")
        _dbg_tensors["sp"] = sp_d
        _dbg_tensors["pt"] = pt_d
        _dbg_tensors["ytm"] = ytm_d
        _dbg_tensors["recip"] = recip_d
        _dbg_tensors["bc"] = bc_d

    with ExitStack() as ctx:
        tc = ctx.enter_context(tile.TileContext(nc))

        const = ctx.enter_context(tc.tile_pool(name="const", bufs=1))
        big = ctx.enter_context(tc.tile_pool(name="big", bufs=1))
        qp2 = ctx.enter_context(tc.tile_pool(name="qp2", bufs=2))
        ptp = ctx.enter_context(tc.tile_pool(
            name="ptp", bufs=(4 if 'ptp4' in opts else 3)))
        ytmp = ctx.enter_context(tc.tile_pool(name="ytmp", bufs=3))
        normp = ctx.enter_context(tc.tile_pool(name="normp", bufs=3))
        outp = ctx.enter_context(tc.tile_pool(name="outp", bufs=3))
        y3 = 'y3' in opts
        mmps = ctx.enter_context(tc.tile_pool(name="mmps",
                                              bufs=(1 if y3 else 2),
                                              space="PSUM"))
        sps = ctx.enter_context(tc.tile_pool(name="sps", bufs=2,
                                             space="PSUM"))
        yps = ctx.enter_context(tc.tile_pool(name="yps",
                                             bufs=(3 if y3 else 2),
                                             space="PSUM"))

        # ---- constants ----
        # tri[k, q] = 1.0 where q >= k else 0 (multiplicative causal mask
        # for the diagonal 128x128 block of an S^T tile)
        tri = const.tile([128, 128], bf16)
        nc.gpsimd.memset(tri, 1.0)
        nc.gpsimd.affine_select(
            out=tri, in_=tri, compare_op=mybir.AluOpType.is_ge,
            fill=0.0, base=0, pattern=[[1, 128]], channel_multiplier=-1,
        )
        ones1 = const.tile([1, 128], bf16)
        nc.gpsimd.memset(ones1, 1.0)

        # qk bias, one column per m-block: bqk_sb[p, mb] = bqkv[mb*128 + p]
        bqk_sb = const.tile([128, 8], f32)
        nc.sync.dma_start(bqk_sb,
                          bqkv_d[0:1024].rearrange("(mb p) -> p mb", p=128))
        bv_f = const.tile([1, 512], f32)
        nc.sync.dma_start(bv_f, bqkv_d[None, 1024:1536])
        bv_sb = const.tile([1, 512], bf16)
        nc.vector.tensor_copy(bv_sb, bv_f)

        # ---- persistent tensors ----
        xT = big.tile([128, NCB, T], bf16, name="xT")
        w_all = big.tile([128, NCB, 1536], bf16, name="w_all")
        wp_sb = big.tile([128, 4, 1024], bf16, name="wp_sb")
        # kdiag mode: qkT is only a DMA staging buffer -> per-segment pool
        qkT = (None if 'kdiag' in opts
               else big.tile([128, 8, T], bf16, name="qkT"))
        v_sb = big.tile([128, H_LOC, NTB, 65], bf16, name="v_sb")
        yt = big.tile([128, 4, T], bf16, name="yt")

        kdiag = qT2 = None
        if 'kdiag' in opts:
            # K=128 attention: block-diagonal K stationaries (one [128,128]
            # tile per head x key-block; off-diagonal quadrants stay zero)
            # and partition-replicated Q, both filled by SBUF->SBUF DMA
            kdiag = big.tile([128, H_LOC, NTB, 128], bf16, name="kdiag")
            nc.gpsimd.memset(kdiag, 0.0)

        nc.gpsimd.memset(v_sb[:, :, :, 64:65], 1.0)

        for _rep in range(reps):
            _emit_v2(nc, tc, mybir, AF, f32, bf16,
                     ptp, ytmp, normp, outp, mmps, sps, yps,
                     xt_d, wqkv_d, wp_d, out_d,
                     xT, w_all, wp_sb, qkT, v_sb, yt,
                     tri, ones1, bqk_sb, bv_sb, phases, opts,
                     kdiag=kdiag, qp2=qp2)
            if dbg:
                nc.sync.dma_start(qkT_d[:, :, :], qkT)
                nc.sync.dma_start(v_d[:, :, :, :], v_sb)
                nc.sync.dma_start(yt_d[:, :, :], yt)

    try:
        nc.compile()
    finally:
        for k, v in _patch.items():
            setattr(hw_specs.TRN2Spec, k, v)
    return nc


def _emit_v2(nc, tc, mybir, AF, f32, bf16,
             ptp, ytmp, normp, outp, mmps, sps, yps,
             xt_d, wqkv_d, wp_d, out_d,
             xT, w_all, wp_sb, qkT, v_sb, yt,
             tri, ones1, bqk_sb, bv_sb, phases, opts,
             kdiag=None, qp2=None):
    use_kd = 'kdiag' in opts
    seg_qT2 = {}
    seg_qk = {}

    def qk_stage(t0):
        if use_kd:
            if t0 not in seg_qk:
                seg_qk[t0] = qp2.tile([128, 8, 512], bf16, name="qks",
                                      tag="qks")
            return seg_qk[t0], 0
        return qkT, t0
    wqk = w_all[:, :, 0:1024]
    wv = w_all[:, :, 1024:1536]
    wqkv_v = wqkv_d.rearrange("(cb p) m -> p cb m", p=128)
    xt_v = xt_d.rearrange("(cb p) t -> p cb t", p=128)

    # ---- upfront DMAs, in first-use order; DMA engines run ahead ----
    nc.sync.dma_start(xT[:, :, 0:512], xt_v[:, :, 0:512])
    nc.sync.dma_start(wv, wqkv_v[:, :, 1024:1536])
    nc.sync.dma_start(wqk, wqkv_v[:, :, 0:1024])
    if not use_kd:
        for ts in range(1, NSEG):
            nc.sync.dma_start(xT[:, :, ts * 512:(ts + 1) * 512],
                              xt_v[:, :, ts * 512:(ts + 1) * 512])
        nc.sync.dma_start(wp_sb, wp_d.rearrange("(pb p) c -> p pb c", p=128))

    def emit_deferred_loads():
        # queued on SP after segment-0 replication DMAs so those go first
        for ts in range(1, NSEG):
            nc.sync.dma_start(xT[:, :, ts * 512:(ts + 1) * 512],
                              xt_v[:, :, ts * 512:(ts + 1) * 512])
        nc.sync.dma_start(wp_sb, wp_d.rearrange("(pb p) c -> p pb c", p=128))

    def emit_v(tb):
        vp = mmps.tile([128, 512], f32, name="vp", tag="mm")
        nobias = 'nobias' in opts
        for cb in range(NCB):
            nc.tensor.matmul(
                vp, xT[:, cb, tb * 128:(tb + 1) * 128],
                wv[:, cb, :], start=(cb == 0), stop=(nobias and cb == NCB - 1))
        if not nobias:
            # bias via K=1 matmul: ones1^T @ bv adds bv to every row
            nc.tensor.matmul(vp, ones1, bv_sb, start=False, stop=True)
        nc.vector.tensor_copy(
            v_sb[:, :, tb, 0:64],
            vp.rearrange("p (h d) -> p h d", h=H_LOC))

    def emit_qk(mb, t0):
        qp = mmps.tile([128, 512], f32, name="qp", tag="mm")
        for cb in range(NCB):
            nc.tensor.matmul(
                qp, wqk[:, cb, mb * 128:(mb + 1) * 128],
                xT[:, cb, t0:t0 + 512],
                start=(cb == 0), stop=(cb == NCB - 1))
        stg, off = qk_stage(t0)
        if 'nobias' in opts:
            nc.vector.tensor_copy(stg[:, mb, off:off + 512], qp)
        else:
            nc.vector.tensor_scalar_add(
                stg[:, mb, off:off + 512], qp, bqk_sb[:, mb:mb + 1])

    def emit_repl(ts, pr):
        # build the K=128 attention operands for segment ts, head pair pr:
        # qT2[*, h, :] = q_h replicated on both partition halves;
        # kdiag[*, h, kb, :] = block-diag([k_h 64-key chunk, next chunk])
        t0 = ts * 512
        if ts not in seg_qT2:
            seg_qT2[ts] = qp2.tile([128, H_LOC, 512], bf16, name="qT2s",
                                   tag="qT2s")
        qT2s = seg_qT2[ts]
        stg, off = qk_stage(t0)
        for i in range(2):
            h = 2 * pr + i
            hs = 64 * i
            qsrc = stg[hs:hs + 64, 2 * pr, off:off + 512]
            # SP HWDGE queue (the Act queue would serialize descriptor
            # writes with the exp stream on the Act sequencer); the bulk
            # input loads are emitted after segment-0 replication so these
            # never wait behind them
            nc.sync.dma_start(qT2s[0:64, h, :], qsrc)
            nc.sync.dma_start(qT2s[64:128, h, :], qsrc)
            kv = stg[hs:hs + 64, 2 * pr + 1, off:off + 512].rearrange(
                "p (j f) -> p j f", j=4)
            nc.sync.dma_start(kdiag[0:64, h, 4 * ts:4 * ts + 4, 0:64],
                              kv[:, :, 0:64])
            nc.sync.dma_start(kdiag[64:128, h, 4 * ts:4 * ts + 4, 64:128],
                              kv[:, :, 64:128])

    def emit_attention(ts, pr):
        q0 = ts * 512
        nkb = 4 * (ts + 1)
        if not use_kd:
            qT0 = qkT[0:64, 2 * pr, :]
            kT0 = qkT[0:64, 2 * pr + 1, :]
            qT1 = qkT[64:128, 2 * pr, :]
            kT1 = qkT[64:128, 2 * pr + 1, :]
        qT2s = seg_qT2.get(ts)
        y0 = yps.tile([65, 512], f32, name="y0", tag="y")
        y1 = yps.tile([65, 512], f32, name="y1", tag="y")

        def emit_av(kb, pt, qlo):
            off = qlo - q0
            for i, y_ps in ((0, y0), (1, y1)):
                nc.tensor.matmul(
                    y_ps[:, off:512], v_sb[:, 2 * pr + i, kb, :],
                    pt[:, i, 0:512 - off],
                    start=(kb == 0), stop=(kb == nkb - 1),
                    skip_group_check=True)

        pending = None
        for kb in range(nkb):
            qlo = max(q0, kb * 128)
            qlen = q0 + 512 - qlo
            sp = sps.tile([128, 2, 512], f32, name="sp")
            if use_kd:
                # K=128 S^T via block-diagonal K stationary + replicated Q
                off = qlo - q0
                nc.tensor.matmul(sp[:, 0, 0:qlen],
                                 kdiag[:, 2 * pr, kb, :],
                                 qT2s[:, 2 * pr, off:off + qlen],
                                 start=True, stop=True)
                nc.tensor.matmul(sp[:, 1, 0:qlen],
                                 kdiag[:, 2 * pr + 1, kb, :],
                                 qT2s[:, 2 * pr + 1, off:off + qlen],
                                 start=True, stop=True)
            else:
                # paired S^T: disjoint PE row groups -> concurrent on HW
                nc.tensor.matmul(sp[:, 0, 0:qlen],
                                 kT0[:, kb * 128:(kb + 1) * 128],
                                 qT0[:, qlo:qlo + qlen],
                                 start=True, stop=True)
                nc.tensor.matmul(sp[:, 1, 0:qlen],
                                 kT1[:, kb * 128:(kb + 1) * 128],
                                 qT1[:, qlo:qlo + qlen],
                                 start=True, stop=True)
            pt = ptp.tile([128, 2, 512], bf16, name="pt")
            if 'flatexp' in opts and qlen == 512:
                nc.scalar.activation(
                    pt.rearrange("p i q -> p (i q)"),
                    sp.rearrange("p i q -> p (i q)"),
                    AF.Exp, scale=0.125)
            else:
                nc.scalar.activation(pt[:, :, 0:qlen], sp[:, :, 0:qlen],
                                     AF.Exp, scale=0.125)
            if kb * 128 >= q0:
                # diagonal block: zero the strictly-upper part
                nc.gpsimd.tensor_mul(pt[:, 0, 0:128], pt[:, 0, 0:128], tri)
                nc.gpsimd.tensor_mul(pt[:, 1, 0:128], pt[:, 1, 0:128], tri)
            if 'dbg' in opts and ts == 0 and pr == 0 and kb == 0:
                stg = ytmp.tile([128, 2, 512], f32, name="spstg",
                                tag="dbgst")
                nc.vector.tensor_copy(stg, sp[:, :, :])
                nc.sync.dma_start(_dbg_tensors["sp"][:, :, :], stg)
                nc.sync.dma_start(_dbg_tensors["pt"][:, :, :], pt[:, :, :])
            if pending is not None:
                emit_av(*pending)
            pending = (kb, pt, qlo)
        emit_av(*pending)

        if 'pairnorm' in opts:
            # batched normalization for the head pair: evacuate both y
            # PSUMs into one [128,512] tile, one recip over both sums
            # rows, one 128-partition multiply
            ytm2 = ytmp.tile([128, 512], f32, name="ytm2", tag="ytm2")
            sums2 = normp.tile([1, 2, 512], f32, name="sums2", tag="sums2")
            nc.vector.tensor_copy(ytm2[0:64, :], y0[0:64, :])
            nc.vector.tensor_copy(sums2[:, 0, :], y0[64:65, :])
            nc.vector.tensor_copy(ytm2[64:128, :], y1[0:64, :])
            nc.vector.tensor_copy(sums2[:, 1, :], y1[64:65, :])
            recip2 = normp.tile([1, 2, 512], f32, name="recip2",
                                tag="recip2")
            nc.vector.reciprocal_approx_fast(
                recip2.rearrange("p i q -> p (i q)"),
                sums2.rearrange("p i q -> p (i q)"))
            bc2 = normp.tile([128, 512], f32, name="bc2", tag="bc2")
            nc.gpsimd.partition_broadcast(bc2[0:64, :], recip2[:, 0, :])
            nc.gpsimd.partition_broadcast(bc2[64:128, :], recip2[:, 1, :])
            nc.vector.tensor_mul(yt[:, pr, q0:q0 + 512], ytm2, bc2)
            return
        # deferred normalization: one copy evacuates PSUM; the sums row is
        # re-staged to a base-0 tile (custom-DVE recip needs base 0), the
        # rest runs from SBUF off the critical path
        for i, (y_ps, po) in enumerate(((y0, 0), (y1, 64))):
            if 'sttnorm' in opts:
                # fused path: y stays in PSUM until bc is ready
                sums = normp.tile([1, 512], f32, name="sums")
                nc.vector.tensor_copy(sums, y_ps[64:65, :])
                recip = normp.tile([1, 512], f32, name="recip")
                nc.vector.reciprocal_approx_fast(recip, sums)
                bc = normp.tile([64, 512], f32, name="bc")
                nc.gpsimd.partition_broadcast(bc, recip)
                nc.vector.scalar_tensor_tensor(
                    out=yt[po:po + 64, pr, q0:q0 + 512], in0=y_ps[0:64, :],
                    scalar=1.0, in1=bc,
                    op0=mybir.AluOpType.mult, op1=mybir.AluOpType.mult)
                continue
            ytm = ytmp.tile([65, 512], f32, name="ytm")
            nc.vector.tensor_copy(ytm, y_ps)
            sums = normp.tile([1, 512], f32, name="sums")
            nc.vector.tensor_copy(sums, ytm[64:65, :])
            recip = normp.tile([1, 512], f32, name="recip")
            nc.vector.reciprocal_approx_fast(recip, sums)
            bc = normp.tile([64, 512], f32, name="bc")
            nc.gpsimd.partition_broadcast(bc, recip)
            if 'dbg' in opts and ts == 0 and pr == 0 and i == 0:
                nc.sync.dma_start(_dbg_tensors["ytm"][:, :], ytm)
                nc.sync.dma_start(_dbg_tensors["recip"][:, :], recip)
                nc.sync.dma_start(_dbg_tensors["bc"][:, :], bc)
            nc.vector.tensor_mul(
                yt[po:po + 64, pr, q0:q0 + 512], ytm[0:64, :], bc)

    def emit_proj(tb):
        o_sb = outp.tile([128, 1024], bf16, name="o_sb")
        for ns in range(2):
            pp = mmps.tile([128, 512], f32, name="pp", tag="mm")
            for p in range(4):
                nc.tensor.matmul(
                    pp, yt[:, p, tb * 128:(tb + 1) * 128],
                    wp_sb[:, p, ns * 512:(ns + 1) * 512],
                    start=(p == 0), stop=(p == 3))
            nc.vector.tensor_copy(o_sb[:, ns * 512:(ns + 1) * 512], pp)
        if 'nostore' not in opts:
            if 'storesync' in opts:
                nc.sync.dma_start(out_d[tb * 128:(tb + 1) * 128, :], o_sb)
            else:
                nc.gpsimd.dma_start(out_d[tb * 128:(tb + 1) * 128, :], o_sb)

    # ---- segment 0 QKV: V first, then per-pr QK so attention(0, pr)
    # can start as soon as its own Q/K blocks land ----
    eager0 = 'eager0' in opts
    if 'B' in phases:
        for tb in range(4):
            emit_v(tb)
        if not eager0:
            if use_kd:
                for pr in range(4):
                    emit_qk(2 * pr, 0)
                    emit_qk(2 * pr + 1, 0)
                    emit_repl(0, pr)
                emit_deferred_loads()
            else:
                for mb in range(8):
                    emit_qk(mb, 0)

    # ---- steady state: attention(ts) with next-segment QKV and
    # previous-segment proj pieces interleaved to fill PE gaps ----
    projend = 'projil' not in opts
    for ts in range(NSEG):
        for pr in range(4):
            if ts == 0 and eager0 and 'B' in phases:
                emit_qk(2 * pr, 0)
                emit_qk(2 * pr + 1, 0)
                if use_kd:
                    emit_repl(0, pr)
            if 'C' in phases:
                emit_attention(ts, pr)
            if ts < NSEG - 1 and 'B' in phases:
                emit_v(4 * (ts + 1) + pr)
                emit_qk(2 * pr, (ts + 1) * 512)
                emit_qk(2 * pr + 1, (ts + 1) * 512)
                if use_kd:
                    emit_repl(ts + 1, pr)
            if not projend and ts > 0 and 'D' in phases:
                emit_proj(4 * (ts - 1) + pr)
        if projend and 'D' in phases:
            for tb in range(4 * ts, 4 * ts + 4):
                emit_proj(tb)
    if not projend and 'D' in phases:
        for tb in range(4 * (NSEG - 1), 4 * NSEG):
            emit_proj(tb)


def _shard_inputs(x, w_attn, b_attn, w_proj):
    """Build per-core input maps (pair-packed q/k layouts; see module doc)."""
    wq = w_attn[:, 0:C].reshape(C, N_HEAD, D)
    wk = w_attn[:, C:2 * C].reshape(C, N_HEAD, D)
    wv = w_attn[:, 2 * C:3 * C].reshape(C, N_HEAD, D)
    bq = b_attn[0:C].reshape(N_HEAD, D)
    bk = b_attn[C:2 * C].reshape(N_HEAD, D)
    bv = b_attn[2 * C:3 * C].reshape(N_HEAD, D)

    xt_by_batch = [
        np.ascontiguousarray(x[b].T).astype(ml_dtypes.bfloat16)
        for b in range(B)
    ]

    in_maps = []
    for core in range(N_CORES):
        b, g = core // 2, core % 2
        h0 = g * H_LOC
        qk_blocks, bqk_parts = [], []
        for p in range(4):
            hA, hB = h0 + 2 * p, h0 + 2 * p + 1
            qk_blocks.append(np.concatenate([wq[:, hA], wq[:, hB]], axis=1))
            qk_blocks.append(np.concatenate([wk[:, hA], wk[:, hB]], axis=1))
            bqk_parts.append(np.concatenate([bq[hA], bq[hB]]))
            bqk_parts.append(np.concatenate([bk[hA], bk[hB]]))
        wqkv = np.concatenate(
            qk_blocks + [wv[:, h0:h0 + H_LOC].reshape(C, H_LOC * D)], axis=1)
        bqkv = np.concatenate(
            bqk_parts + [bv[h0:h0 + H_LOC].reshape(H_LOC * D)])
        wproj = w_proj.reshape(N_HEAD, D, C)[h0:h0 + H_LOC].reshape(
            H_LOC * D, C)
        in_maps.append({
            "xt": xt_by_batch[b],
            "wqkv": np.ascontiguousarray(wqkv).astype(ml_dtypes.bfloat16),
            "bqkv": np.ascontiguousarray(bqkv, dtype=np.float32),
            "wproj": np.ascontiguousarray(wproj).astype(ml_dtypes.bfloat16),
        })
    return in_maps


def kernel(x, w_attn, b_attn, w_proj, b_proj):
    global last_exec_ns
    from concourse.bass_utils import run_bass_kernel_spmd

    x = np.asarray(x, dtype=np.float32)
    w_attn = np.asarray(w_attn, dtype=np.float32)
    b_attn = np.asarray(b_attn, dtype=np.float32)
    w_proj = np.asarray(w_proj, dtype=np.float32)
    b_proj = np.asarray(b_proj, dtype=np.float32)

    kopts = tuple(o for o in os.environ.get("KOPTS", "").split(",") if o)
    if _cache.get("opts") != kopts:
        _cache["nc"] = _build_program(opts=kopts)
        _cache["opts"] = kopts
    nc = _cache["nc"]

    in_maps = _shard_inputs(x, w_attn, b_attn, w_proj)
    trace = os.environ.get("KERNEL_TRACE", "0") == "1"
    if trace:
        try:
            import antenv.axon_hooks  # noqa: F401
        except ImportError:
            trace = False
    res = run_bass_kernel_spmd(nc, in_maps, core_ids=list(range(N_CORES)),
                               trace=trace)
    last_exec_ns = res.exec_time_ns

    out = np.empty((B, T, C), dtype=np.float32)
    for b in range(B):
        out[b] = (res.results[2 * b]["out"].astype(np.float32)
                  + res.results[2 * b + 1]["out"].astype(np.float32)
                  + b_proj[None, :])
    return out



# revision 21
# speedup vs baseline: 1.0322x; 1.0322x over previous
"""Trainium2 Bass kernel: GPT-style causal self-attention block.

Computes, for x[B=4, T=2048, C=1024], 16 heads x 64 dims:
    qkv = x @ w_attn + b_attn ; causal softmax attention ; y @ w_proj + b_proj

Sharding (8 cores): data-parallel over B (4) x tensor-parallel over head
groups (2 groups of 8 heads, Megatron style).  Each core:
  - receives x^T (host-transposed) and its slice of the weights,
  - computes Q^T/K^T (head-pair packed on partitions) and token-major V,
  - runs causal attention per head-pair: the two heads' S^T matmuls sit on
    disjoint PE row groups (partitions 0-63 / 64-127) so they execute
    concurrently on the 128x128 array; one ScalarE exp instruction covers
    both heads' tiles; AV matmuls carry a ones-column so the softmax
    denominators fall out of the same accumulation,
  - normalization is deferred off the PSUM critical path (single DVE copy
    evacuates y+sums, then recip/broadcast/scale from SBUF),
  - applies its row-slice of w_proj (row-parallel) producing a partial
    [T, C] output.  Host sums the two partials per batch and adds b_proj.

The per-512-token-segment loop interleaves QKV -> attention -> proj so the
TensorE-heavy projection work overlaps the ScalarE-heavy softmax work.
"""

import os
import ml_dtypes
import numpy as np

B, T, C = 4, 2048, 1024
N_HEAD = 16
D = 64  # head dim
H_LOC = 8  # heads per core
N_CORES = 8

NTB = T // 128   # 16 token blocks
NCB = C // 128   # 8 contraction blocks
NSEG = T // 512  # 4 token segments
QQ = 512         # attention q-tile width

_cache = {}
_dbg_tensors = {}

last_exec_ns = None


def _build_program(reps=1, phases='ABCD', opts=()):
    from contextlib import ExitStack

    import concourse.bass as bass
    import concourse.mybir as mybir
    import concourse.tile as tile
    from concourse import bacc

    f32 = mybir.dt.float32
    bf16 = mybir.dt.bfloat16
    AF = mybir.ActivationFunctionType

    import concourse.hw_specs as hw_specs
    _patch = {}
    if 'pe32' in opts:
        # scheduling-only hint: measured HW bf16 matmul throughput is
        # ~3.24 G cols/s in steady state (mm512x8 microbench: 158 ns per
        # 512-col K=128 matmul); restored before return
        _patch = {"PE_CYCLE": hw_specs.TRN2Spec.PE_CYCLE,
                  "PE_CYCLE_PSTATE_MID": hw_specs.TRN2Spec.PE_CYCLE_PSTATE_MID}
        hw_specs.TRN2Spec.PE_CYCLE = 1e9 / 3.24e9
        hw_specs.TRN2Spec.PE_CYCLE_PSTATE_MID = 1e9 / 3.24e9
    elif 'fastpe' in opts:
        # scheduling-only hint: match the cost model to measured HW matmul
        # throughput (bf16 ~4x the default model) while building; restored
        # before return so no global state leaks
        _patch = {"PE_CYCLE": hw_specs.TRN2Spec.PE_CYCLE,
                  "PE_CYCLE_PSTATE_MID": hw_specs.TRN2Spec.PE_CYCLE_PSTATE_MID}
        hw_specs.TRN2Spec.PE_CYCLE = 1e9 / 9.6e9
        hw_specs.TRN2Spec.PE_CYCLE_PSTATE_MID = 1e9 / 4.8e9

    nc = bacc.Bacc("TRN2", target_bir_lowering=False, debug=False,
                   num_devices=N_CORES)

    xt_d = nc.dram_tensor("xt", [C, T], bf16, kind="ExternalInput")
    wqkv_d = nc.dram_tensor("wqkv", [C, 1536], bf16, kind="ExternalInput")
    bqkv_d = nc.dram_tensor("bqkv", [1536], f32, kind="ExternalInput")
    wp_d = nc.dram_tensor("wproj", [512, C], bf16, kind="ExternalInput")
    out_d = nc.dram_tensor("out", [T, C], bf16, kind="ExternalOutput")
    dbg = 'dbg' in opts
    if dbg:
        qkT_d = nc.dram_tensor("qkT_dbg", [128, 8, T], bf16,
                               kind="ExternalOutput")
        v_d = nc.dram_tensor("v_dbg", [128, H_LOC, NTB, 65], bf16,
                             kind="ExternalOutput")
        yt_d = nc.dram_tensor("yt_dbg", [128, 4, T], bf16,
                              kind="ExternalOutput")
        sp_d = nc.dram_tensor("sp_dbg", [128, 2, 512], f32,
                              kind="ExternalOutput")
        pt_d = nc.dram_tensor("pt_dbg", [128, 2, 512], bf16,
                              kind="ExternalOutput")
        ytm_d = nc.dram_tensor("ytm_dbg", [65, 512], f32,
                               kind="ExternalOutput")
        recip_d = nc.dram_tensor("recip_dbg", [1, 512], f32,
                                 kind="ExternalOutput")
        bc_d = nc.dram_tensor("bc_dbg", [64, 512], f32,
                              kind="ExternalOutput")
        _dbg_tensors["sp"] = sp_d
        _dbg_tensors["pt"] = pt_d
        _dbg_tensors["ytm"] = ytm_d
        _dbg_tensors["recip"] = recip_d
        _dbg_tensors["bc"] = bc_d

    with ExitStack() as ctx:
        tc = ctx.enter_context(tile.TileContext(nc))

        const = ctx.enter_context(tc.tile_pool(name="const", bufs=1))
        big = ctx.enter_context(tc.tile_pool(name="big", bufs=1))
        qp2 = ctx.enter_context(tc.tile_pool(name="qp2", bufs=2))
        ptp = ctx.enter_context(tc.tile_pool(
            name="ptp", bufs=(4 if 'ptp4' in opts else 3)))
        ytmp = ctx.enter_context(tc.tile_pool(name="ytmp", bufs=3))
        normp = ctx.enter_context(tc.tile_pool(name="normp", bufs=3))
        outp = ctx.enter_context(tc.tile_pool(name="outp", bufs=3))
        y3 = 'y3' in opts
        mmps = ctx.enter_context(tc.tile_pool(name="mmps",
                                              bufs=(1 if y3 else 2),
                                              space="PSUM"))
        sps = ctx.enter_context(tc.tile_pool(name="sps", bufs=2,
                                             space="PSUM"))
        yps = ctx.enter_context(tc.tile_pool(name="yps",
                                             bufs=(3 if y3 else 2),
                                             space="PSUM"))

        # ---- constants ----
        # tri[k, q] = 1.0 where q >= k else 0 (multiplicative causal mask
        # for the diagonal 128x128 block of an S^T tile)
        tri = const.tile([128, 128], bf16)
        nc.gpsimd.memset(tri, 1.0)
        nc.gpsimd.affine_select(
            out=tri, in_=tri, compare_op=mybir.AluOpType.is_ge,
            fill=0.0, base=0, pattern=[[1, 128]], channel_multiplier=-1,
        )
        ones1 = const.tile([1, 128], bf16)
        nc.gpsimd.memset(ones1, 1.0)

        # qk bias, one column per m-block: bqk_sb[p, mb] = bqkv[mb*128 + p]
        bqk_sb = const.tile([128, 8], f32)
        nc.sync.dma_start(bqk_sb,
                          bqkv_d[0:1024].rearrange("(mb p) -> p mb", p=128))
        bv_f = const.tile([1, 512], f32)
        nc.sync.dma_start(bv_f, bqkv_d[None, 1024:1536])
        bv_sb = const.tile([1, 512], bf16)
        nc.vector.tensor_copy(bv_sb, bv_f)

        # ---- persistent tensors ----
        xT = big.tile([128, NCB, T], bf16, name="xT")
        w_all = big.tile([128, NCB, 1536], bf16, name="w_all")
        wp_sb = big.tile([128, 4, 1024], bf16, name="wp_sb")
        # kdiag mode: qkT is only a DMA staging buffer -> per-segment pool
        qkT = (None if 'kdiag' in opts
               else big.tile([128, 8, T], bf16, name="qkT"))
        v_sb = big.tile([128, H_LOC, NTB, 65], bf16, name="v_sb")
        yt = big.tile([128, 4, T], bf16, name="yt")

        kdiag = qT2 = None
        if 'kdiag' in opts:
            # K=128 attention: block-diagonal K stationaries (one [128,128]
            # tile per head x key-block; off-diagonal quadrants stay zero)
            # and partition-replicated Q, both filled by SBUF->SBUF DMA
            kdiag = big.tile([128, H_LOC, NTB, 128], bf16, name="kdiag")
            nc.gpsimd.memset(kdiag, 0.0)

        nc.gpsimd.memset(v_sb[:, :, :, 64:65], 1.0)

        for _rep in range(reps):
            _emit_v2(nc, tc, mybir, AF, f32, bf16,
                     ptp, ytmp, normp, outp, mmps, sps, yps,
                     xt_d, wqkv_d, wp_d, out_d,
                     xT, w_all, wp_sb, qkT, v_sb, yt,
                     tri, ones1, bqk_sb, bv_sb, phases, opts,
                     kdiag=kdiag, qp2=qp2)
            if dbg:
                nc.sync.dma_start(qkT_d[:, :, :], qkT)
                nc.sync.dma_start(v_d[:, :, :, :], v_sb)
                nc.sync.dma_start(yt_d[:, :, :], yt)

    try:
        nc.compile()
    finally:
        for k, v in _patch.items():
            setattr(hw_specs.TRN2Spec, k, v)
    return nc


def _emit_v2(nc, tc, mybir, AF, f32, bf16,
             ptp, ytmp, normp, outp, mmps, sps, yps,
             xt_d, wqkv_d, wp_d, out_d,
             xT, w_all, wp_sb, qkT, v_sb, yt,
             tri, ones1, bqk_sb, bv_sb, phases, opts,
             kdiag=None, qp2=None):
    use_kd = 'kdiag' in opts
    seg_qT2 = {}
    seg_qk = {}

    def qk_stage(t0):
        if use_kd:
            if t0 not in seg_qk:
                seg_qk[t0] = qp2.tile([128, 8, 512], bf16, name="qks",
                                      tag="qks")
            return seg_qk[t0], 0
        return qkT, t0
    wqk = w_all[:, :, 0:1024]
    wv = w_all[:, :, 1024:1536]
    wqkv_v = wqkv_d.rearrange("(cb p) m -> p cb m", p=128)
    xt_v = xt_d.rearrange("(cb p) t -> p cb t", p=128)

    # ---- upfront DMAs, in first-use order; DMA engines run ahead ----
    nc.sync.dma_start(xT[:, :, 0:512], xt_v[:, :, 0:512])
    nc.sync.dma_start(wv, wqkv_v[:, :, 1024:1536])
    nc.sync.dma_start(wqk, wqkv_v[:, :, 0:1024])
    for ts in range(1, NSEG):
        nc.sync.dma_start(xT[:, :, ts * 512:(ts + 1) * 512],
                          xt_v[:, :, ts * 512:(ts + 1) * 512])
    nc.sync.dma_start(wp_sb, wp_d.rearrange("(pb p) c -> p pb c", p=128))

    def emit_v(tb):
        vp = mmps.tile([128, 512], f32, name="vp", tag="mm")
        nobias = 'nobias' in opts
        for cb in range(NCB):
            nc.tensor.matmul(
                vp, xT[:, cb, tb * 128:(tb + 1) * 128],
                wv[:, cb, :], start=(cb == 0), stop=(nobias and cb == NCB - 1))
        if not nobias:
            # bias via K=1 matmul: ones1^T @ bv adds bv to every row
            nc.tensor.matmul(vp, ones1, bv_sb, start=False, stop=True)
        nc.vector.tensor_copy(
            v_sb[:, :, tb, 0:64],
            vp.rearrange("p (h d) -> p h d", h=H_LOC))

    def emit_qk(mb, t0):
        qp = mmps.tile([128, 512], f32, name="qp", tag="mm")
        for cb in range(NCB):
            nc.tensor.matmul(
                qp, wqk[:, cb, mb * 128:(mb + 1) * 128],
                xT[:, cb, t0:t0 + 512],
                start=(cb == 0), stop=(cb == NCB - 1))
        stg, off = qk_stage(t0)
        if 'nobias' in opts:
            nc.vector.tensor_copy(stg[:, mb, off:off + 512], qp)
        else:
            nc.vector.tensor_scalar_add(
                stg[:, mb, off:off + 512], qp, bqk_sb[:, mb:mb + 1])

    def emit_repl_seg(ts):
        # build the K=128 attention operands for the whole segment in 8
        # batched DMAs on the Act HWDGE queue (distinct from the SP queue
        # carrying bulk input loads; batching keeps the Act SEQ trigger
        # cost low so it never gates the exp stream)
        t0 = ts * 512
        if ts not in seg_qT2:
            seg_qT2[ts] = qp2.tile([128, H_LOC, 512], bf16, name="qT2s",
                                   tag="qT2s")
        qT2s = seg_qT2[ts]
        stg, off = qk_stage(t0)
        dma_eng = nc.sync if 'replsp' in opts else nc.scalar
        for i in range(2):            # source partition half (head parity)
            hs = 64 * i
            # q features of the 4 pairs: stg dim-1 indices 0,2,4,6
            qsrc = stg[hs:hs + 64, :, off:off + 512].rearrange(
                "p (pr c) q -> p c pr q", c=2)[:, 0]
            for dst_half in range(2):
                d0 = 64 * dst_half
                qdst = qT2s[d0:d0 + 64, :, :].rearrange(
                    "p (pr c) q -> p c pr q", c=2)[:, i]
                dma_eng.dma_start(qdst, qsrc)
            # k features (stg dim-1 indices 1,3,5,7), 64-key chunks j;
            # DMA APs are limited to 3 dims, so kdiag goes per head
            for pr in range(4):
                h = 2 * pr + i
                kv = stg[hs:hs + 64, 2 * pr + 1, off:off + 512].rearrange(
                    "p (j f) -> p j f", j=4)
                dma_eng.dma_start(
                    kdiag[0:64, h, 4 * ts:4 * ts + 4, 0:64], kv[:, :, 0:64])
                dma_eng.dma_start(
                    kdiag[64:128, h, 4 * ts:4 * ts + 4, 64:128],
                    kv[:, :, 64:128])

    def emit_attention(ts, pr):
        q0 = ts * 512
        nkb = 4 * (ts + 1)
        if not use_kd:
            qT0 = qkT[0:64, 2 * pr, :]
            kT0 = qkT[0:64, 2 * pr + 1, :]
            qT1 = qkT[64:128, 2 * pr, :]
            kT1 = qkT[64:128, 2 * pr + 1, :]
        qT2s = seg_qT2.get(ts)
        y0 = yps.tile([65, 512], f32, name="y0", tag="y")
        y1 = yps.tile([65, 512], f32, name="y1", tag="y")

        def emit_av(kb, pt, qlo):
            off = qlo - q0
            for i, y_ps in ((0, y0), (1, y1)):
                nc.tensor.matmul(
                    y_ps[:, off:512], v_sb[:, 2 * pr + i, kb, :],
                    pt[:, i, 0:512 - off],
                    start=(kb == 0), stop=(kb == nkb - 1),
                    skip_group_check=True)

        pending = None
        for kb in range(nkb):
            qlo = max(q0, kb * 128)
            qlen = q0 + 512 - qlo
            sp = sps.tile([128, 2, 512], f32, name="sp")
            if use_kd:
                # K=128 S^T via block-diagonal K stationary + replicated Q
                off = qlo - q0
                nc.tensor.matmul(sp[:, 0, 0:qlen],
                                 kdiag[:, 2 * pr, kb, :],
                                 qT2s[:, 2 * pr, off:off + qlen],
                                 start=True, stop=True)
                nc.tensor.matmul(sp[:, 1, 0:qlen],
                                 kdiag[:, 2 * pr + 1, kb, :],
                                 qT2s[:, 2 * pr + 1, off:off + qlen],
                                 start=True, stop=True)
            else:
                # paired S^T: disjoint PE row groups -> concurrent on HW
                nc.tensor.matmul(sp[:, 0, 0:qlen],
                                 kT0[:, kb * 128:(kb + 1) * 128],
                                 qT0[:, qlo:qlo + qlen],
                                 start=True, stop=True)
                nc.tensor.matmul(sp[:, 1, 0:qlen],
                                 kT1[:, kb * 128:(kb + 1) * 128],
                                 qT1[:, qlo:qlo + qlen],
                                 start=True, stop=True)
            pt = ptp.tile([128, 2, 512], bf16, name="pt")
            if 'flatexp' in opts and qlen == 512:
                nc.scalar.activation(
                    pt.rearrange("p i q -> p (i q)"),
                    sp.rearrange("p i q -> p (i q)"),
                    AF.Exp, scale=0.125)
            else:
                nc.scalar.activation(pt[:, :, 0:qlen], sp[:, :, 0:qlen],
                                     AF.Exp, scale=0.125)
            if kb * 128 >= q0:
                # diagonal block: zero the strictly-upper part
                nc.gpsimd.tensor_mul(pt[:, 0, 0:128], pt[:, 0, 0:128], tri)
                nc.gpsimd.tensor_mul(pt[:, 1, 0:128], pt[:, 1, 0:128], tri)
            if 'dbg' in opts and ts == 0 and pr == 0 and kb == 0:
                stg = ytmp.tile([128, 2, 512], f32, name="spstg",
                                tag="dbgst")
                nc.vector.tensor_copy(stg, sp[:, :, :])
                nc.sync.dma_start(_dbg_tensors["sp"][:, :, :], stg)
                nc.sync.dma_start(_dbg_tensors["pt"][:, :, :], pt[:, :, :])
            if pending is not None:
                emit_av(*pending)
            pending = (kb, pt, qlo)
        emit_av(*pending)

        if 'pairnorm' in opts:
            # batched normalization for the head pair: evacuate both y
            # PSUMs into one [128,512] tile, one recip over both sums
            # rows, one 128-partition multiply
            ytm2 = ytmp.tile([128, 512], f32, name="ytm2", tag="ytm2")
            sums2 = normp.tile([1, 2, 512], f32, name="sums2", tag="sums2")
            nc.vector.tensor_copy(ytm2[0:64, :], y0[0:64, :])
            nc.vector.tensor_copy(sums2[:, 0, :], y0[64:65, :])
            nc.vector.tensor_copy(ytm2[64:128, :], y1[0:64, :])
            nc.vector.tensor_copy(sums2[:, 1, :], y1[64:65, :])
            recip2 = normp.tile([1, 2, 512], f32, name="recip2",
                                tag="recip2")
            nc.vector.reciprocal_approx_fast(
                recip2.rearrange("p i q -> p (i q)"),
                sums2.rearrange("p i q -> p (i q)"))
            bc2 = normp.tile([128, 512], f32, name="bc2", tag="bc2")
            nc.gpsimd.partition_broadcast(bc2[0:64, :], recip2[:, 0, :])
            nc.gpsimd.partition_broadcast(bc2[64:128, :], recip2[:, 1, :])
            nc.vector.tensor_mul(yt[:, pr, q0:q0 + 512], ytm2, bc2)
            return
        # deferred normalization: one copy evacuates PSUM; the sums row is
        # re-staged to a base-0 tile (custom-DVE recip needs base 0), the
        # rest runs from SBUF off the critical path
        for i, (y_ps, po) in enumerate(((y0, 0), (y1, 64))):
            if 'sttnorm' in opts:
                # fused path: y stays in PSUM until bc is ready
                sums = normp.tile([1, 512], f32, name="sums")
                nc.vector.tensor_copy(sums, y_ps[64:65, :])
                recip = normp.tile([1, 512], f32, name="recip")
                nc.vector.reciprocal_approx_fast(recip, sums)
                bc = normp.tile([64, 512], f32, name="bc")
                nc.gpsimd.partition_broadcast(bc, recip)
                nc.vector.scalar_tensor_tensor(
                    out=yt[po:po + 64, pr, q0:q0 + 512], in0=y_ps[0:64, :],
                    scalar=1.0, in1=bc,
                    op0=mybir.AluOpType.mult, op1=mybir.AluOpType.mult)
                continue
            ytm = ytmp.tile([65, 512], f32, name="ytm")
            nc.vector.tensor_copy(ytm, y_ps)
            sums = normp.tile([1, 512], f32, name="sums")
            nc.vector.tensor_copy(sums, ytm[64:65, :])
            recip = normp.tile([1, 512], f32, name="recip")
            nc.vector.reciprocal_approx_fast(recip, sums)
            bc = normp.tile([64, 512], f32, name="bc")
            nc.gpsimd.partition_broadcast(bc, recip)
            if 'dbg' in opts and ts == 0 and pr == 0 and i == 0:
                nc.sync.dma_start(_dbg_tensors["ytm"][:, :], ytm)
                nc.sync.dma_start(_dbg_tensors["recip"][:, :], recip)
                nc.sync.dma_start(_dbg_tensors["bc"][:, :], bc)
            nc.vector.tensor_mul(
                yt[po:po + 64, pr, q0:q0 + 512], ytm[0:64, :], bc)

    def emit_proj(tb):
        o_sb = outp.tile([128, 1024], bf16, name="o_sb")
        for ns in range(2):
            pp = mmps.tile([128, 512], f32, name="pp", tag="mm")
            for p in range(4):
                nc.tensor.matmul(
                    pp, yt[:, p, tb * 128:(tb + 1) * 128],
                    wp_sb[:, p, ns * 512:(ns + 1) * 512],
                    start=(p == 0), stop=(p == 3))
            nc.vector.tensor_copy(o_sb[:, ns * 512:(ns + 1) * 512], pp)
        if 'nostore' not in opts:
            if 'storesync' in opts:
                nc.sync.dma_start(out_d[tb * 128:(tb + 1) * 128, :], o_sb)
            else:
                nc.gpsimd.dma_start(out_d[tb * 128:(tb + 1) * 128, :], o_sb)

    # ---- segment 0 QKV: V first, then per-pr QK so attention(0, pr)
    # can start as soon as its own Q/K blocks land ----
    eager0 = 'eager0' in opts
    if 'B' in phases:
        for tb in range(4):
            emit_v(tb)
        if not eager0:
            for mb in range(8):
                emit_qk(mb, 0)
            if use_kd:
                emit_repl_seg(0)

    # ---- steady state: attention(ts) with next-segment QKV and
    # previous-segment proj pieces interleaved to fill PE gaps ----
    projend = 'projil' not in opts
    for ts in range(NSEG):
        for pr in range(4):
            if ts == 0 and eager0 and 'B' in phases:
                emit_qk(2 * pr, 0)
                emit_qk(2 * pr + 1, 0)
                if use_kd and pr == 3:
                    emit_repl_seg(0)
            if 'C' in phases:
                emit_attention(ts, pr)
            if ts < NSEG - 1 and 'B' in phases:
                emit_v(4 * (ts + 1) + pr)
                emit_qk(2 * pr, (ts + 1) * 512)
                emit_qk(2 * pr + 1, (ts + 1) * 512)
                if use_kd and pr == 3:
                    emit_repl_seg(ts + 1)
            if not projend and ts > 0 and 'D' in phases:
                emit_proj(4 * (ts - 1) + pr)
        if projend and 'D' in phases:
            for tb in range(4 * ts, 4 * ts + 4):
                emit_proj(tb)
    if not projend and 'D' in phases:
        for tb in range(4 * (NSEG - 1), 4 * NSEG):
            emit_proj(tb)


def _shard_inputs(x, w_attn, b_attn, w_proj):
    """Build per-core input maps (pair-packed q/k layouts; see module doc)."""
    wq = w_attn[:, 0:C].reshape(C, N_HEAD, D)
    wk = w_attn[:, C:2 * C].reshape(C, N_HEAD, D)
    wv = w_attn[:, 2 * C:3 * C].reshape(C, N_HEAD, D)
    bq = b_attn[0:C].reshape(N_HEAD, D)
    bk = b_attn[C:2 * C].reshape(N_HEAD, D)
    bv = b_attn[2 * C:3 * C].reshape(N_HEAD, D)

    xt_by_batch = [
        np.ascontiguousarray(x[b].T).astype(ml_dtypes.bfloat16)
        for b in range(B)
    ]

    in_maps = []
    for core in range(N_CORES):
        b, g = core // 2, core % 2
        h0 = g * H_LOC
        qk_blocks, bqk_parts = [], []
        for p in range(4):
            hA, hB = h0 + 2 * p, h0 + 2 * p + 1
            qk_blocks.append(np.concatenate([wq[:, hA], wq[:, hB]], axis=1))
            qk_blocks.append(np.concatenate([wk[:, hA], wk[:, hB]], axis=1))
            bqk_parts.append(np.concatenate([bq[hA], bq[hB]]))
            bqk_parts.append(np.concatenate([bk[hA], bk[hB]]))
        wqkv = np.concatenate(
            qk_blocks + [wv[:, h0:h0 + H_LOC].reshape(C, H_LOC * D)], axis=1)
        bqkv = np.concatenate(
            bqk_parts + [bv[h0:h0 + H_LOC].reshape(H_LOC * D)])
        wproj = w_proj.reshape(N_HEAD, D, C)[h0:h0 + H_LOC].reshape(
            H_LOC * D, C)
        in_maps.append({
            "xt": xt_by_batch[b],
            "wqkv": np.ascontiguousarray(wqkv).astype(ml_dtypes.bfloat16),
            "bqkv": np.ascontiguousarray(bqkv, dtype=np.float32),
            "wproj": np.ascontiguousarray(wproj).astype(ml_dtypes.bfloat16),
        })
    return in_maps


def kernel(x, w_attn, b_attn, w_proj, b_proj):
    global last_exec_ns
    from concourse.bass_utils import run_bass_kernel_spmd

    x = np.asarray(x, dtype=np.float32)
    w_attn = np.asarray(w_attn, dtype=np.float32)
    b_attn = np.asarray(b_attn, dtype=np.float32)
    w_proj = np.asarray(w_proj, dtype=np.float32)
    b_proj = np.asarray(b_proj, dtype=np.float32)

    kopts = tuple(o for o in os.environ.get("KOPTS", "").split(",") if o)
    if _cache.get("opts") != kopts:
        _cache["nc"] = _build_program(opts=kopts)
        _cache["opts"] = kopts
    nc = _cache["nc"]

    in_maps = _shard_inputs(x, w_attn, b_attn, w_proj)
    trace = os.environ.get("KERNEL_TRACE", "0") == "1"
    if trace:
        try:
            import antenv.axon_hooks  # noqa: F401
        except ImportError:
            trace = False
    res = run_bass_kernel_spmd(nc, in_maps, core_ids=list(range(N_CORES)),
                               trace=trace)
    last_exec_ns = res.exec_time_ns

    out = np.empty((B, T, C), dtype=np.float32)
    for b in range(B):
        out[b] = (res.results[2 * b]["out"].astype(np.float32)
                  + res.results[2 * b + 1]["out"].astype(np.float32)
                  + b_proj[None, :])
    return out



# revision 23
# speedup vs baseline: 1.0702x; 1.0368x over previous
"""Trainium2 Bass kernel: GPT-style causal self-attention block.

Computes, for x[B=4, T=2048, C=1024], 16 heads x 64 dims:
    qkv = x @ w_attn + b_attn ; causal softmax attention ; y @ w_proj + b_proj

Sharding (8 cores): data-parallel over B (4) x tensor-parallel over head
groups (2 groups of 8 heads, Megatron style).  Each core:
  - receives x^T (host-transposed) and its slice of the weights,
  - computes Q^T/K^T (head-pair packed on partitions) and token-major V,
  - runs causal attention per head-pair: the two heads' S^T matmuls sit on
    disjoint PE row groups (partitions 0-63 / 64-127) so they execute
    concurrently on the 128x128 array; one ScalarE exp instruction covers
    both heads' tiles; AV matmuls carry a ones-column so the softmax
    denominators fall out of the same accumulation,
  - normalization is deferred off the PSUM critical path (single DVE copy
    evacuates y+sums, then recip/broadcast/scale from SBUF),
  - applies its row-slice of w_proj (row-parallel) producing a partial
    [T, C] output.  Host sums the two partials per batch and adds b_proj.

The per-512-token-segment loop interleaves QKV -> attention -> proj so the
TensorE-heavy projection work overlaps the ScalarE-heavy softmax work.
"""

import os
import ml_dtypes
import numpy as np

B, T, C = 4, 2048, 1024
N_HEAD = 16
D = 64  # head dim
H_LOC = 8  # heads per core
N_CORES = 8

NTB = T // 128   # 16 token blocks
NCB = C // 128   # 8 contraction blocks
NSEG = T // 512  # 4 token segments
QQ = 512         # attention q-tile width

_cache = {}
_dbg_tensors = {}

last_exec_ns = None


def _build_program(reps=1, phases='ABCD', opts=()):
    from contextlib import ExitStack

    import concourse.bass as bass
    import concourse.mybir as mybir
    import concourse.tile as tile
    from concourse import bacc

    f32 = mybir.dt.float32
    bf16 = mybir.dt.bfloat16
    AF = mybir.ActivationFunctionType

    import concourse.hw_specs as hw_specs
    _patch = {}
    if 'pe32' in opts:
        # scheduling-only hint: measured HW bf16 matmul throughput is
        # ~3.24 G cols/s in steady state (mm512x8 microbench: 158 ns per
        # 512-col K=128 matmul); restored before return
        _patch = {"PE_CYCLE": hw_specs.TRN2Spec.PE_CYCLE,
                  "PE_CYCLE_PSTATE_MID": hw_specs.TRN2Spec.PE_CYCLE_PSTATE_MID}
        hw_specs.TRN2Spec.PE_CYCLE = 1e9 / 3.24e9
        hw_specs.TRN2Spec.PE_CYCLE_PSTATE_MID = 1e9 / 3.24e9
    elif 'fastpe' in opts:
        # scheduling-only hint: match the cost model to measured HW matmul
        # throughput (bf16 ~4x the default model) while building; restored
        # before return so no global state leaks
        _patch = {"PE_CYCLE": hw_specs.TRN2Spec.PE_CYCLE,
                  "PE_CYCLE_PSTATE_MID": hw_specs.TRN2Spec.PE_CYCLE_PSTATE_MID}
        hw_specs.TRN2Spec.PE_CYCLE = 1e9 / 9.6e9
        hw_specs.TRN2Spec.PE_CYCLE_PSTATE_MID = 1e9 / 4.8e9

    nc = bacc.Bacc("TRN2", target_bir_lowering=False, debug=False,
                   num_devices=N_CORES)

    xt_d = nc.dram_tensor("xt", [C, T], bf16, kind="ExternalInput")
    wqkv_d = nc.dram_tensor("wqkv", [C, 1536], bf16, kind="ExternalInput")
    bqkv_d = nc.dram_tensor("bqkv", [1536], f32, kind="ExternalInput")
    wp_d = nc.dram_tensor("wproj", [512, C], bf16, kind="ExternalInput")
    out_d = nc.dram_tensor("out", [T, C], bf16, kind="ExternalOutput")
    dbg = 'dbg' in opts
    if dbg:
        qkT_d = nc.dram_tensor("qkT_dbg", [128, 8, T], bf16,
                               kind="ExternalOutput")
        v_d = nc.dram_tensor("v_dbg", [128, H_LOC, NTB, 65], bf16,
                             kind="ExternalOutput")
        yt_d = nc.dram_tensor("yt_dbg", [128, 4, T], bf16,
                              kind="ExternalOutput")
        sp_d = nc.dram_tensor("sp_dbg", [128, 2, 512], f32,
                              kind="ExternalOutput")
        pt_d = nc.dram_tensor("pt_dbg", [128, 2, 512], bf16,
                              kind="ExternalOutput")
        ytm_d = nc.dram_tensor("ytm_dbg", [65, 512], f32,
                               kind="ExternalOutput")
        recip_d = nc.dram_tensor("recip_dbg", [1, 512], f32,
                                 kind="ExternalOutput")
        bc_d = nc.dram_tensor("bc_dbg", [64, 512], f32,
                              kind="ExternalOutput")
        _dbg_tensors["sp"] = sp_d
        _dbg_tensors["pt"] = pt_d
        _dbg_tensors["ytm"] = ytm_d
        _dbg_tensors["recip"] = recip_d
        _dbg_tensors["bc"] = bc_d

    with ExitStack() as ctx:
        tc = ctx.enter_context(tile.TileContext(nc))

        const = ctx.enter_context(tc.tile_pool(name="const", bufs=1))
        big = ctx.enter_context(tc.tile_pool(name="big", bufs=1))
        qp2 = ctx.enter_context(tc.tile_pool(name="qp2", bufs=2))
        ptp = ctx.enter_context(tc.tile_pool(
            name="ptp", bufs=(4 if 'ptp4' in opts else 3)))
        ytmp = ctx.enter_context(tc.tile_pool(name="ytmp", bufs=3))
        normp = ctx.enter_context(tc.tile_pool(name="normp", bufs=3))
        outp = ctx.enter_context(tc.tile_pool(name="outp", bufs=3))
        y3 = 'y3' in opts
        mmps = ctx.enter_context(tc.tile_pool(name="mmps",
                                              bufs=(1 if y3 else 2),
                                              space="PSUM"))
        sps = ctx.enter_context(tc.tile_pool(name="sps", bufs=2,
                                             space="PSUM"))
        yps = ctx.enter_context(tc.tile_pool(name="yps",
                                             bufs=(3 if y3 else 2),
                                             space="PSUM"))

        # ---- constants ----
        # tri[k, q] = 1.0 where q >= k else 0 (multiplicative causal mask
        # for the diagonal 128x128 block of an S^T tile)
        tri = const.tile([128, 128], bf16)
        nc.gpsimd.memset(tri, 1.0)
        nc.gpsimd.affine_select(
            out=tri, in_=tri, compare_op=mybir.AluOpType.is_ge,
            fill=0.0, base=0, pattern=[[1, 128]], channel_multiplier=-1,
        )
        ones1 = const.tile([1, 128], bf16)
        nc.gpsimd.memset(ones1, 1.0)

        # qk bias, one column per m-block: bqk_sb[p, mb] = bqkv[mb*128 + p]
        bqk_sb = const.tile([128, 8], f32)
        nc.sync.dma_start(bqk_sb,
                          bqkv_d[0:1024].rearrange("(mb p) -> p mb", p=128))
        bv_f = const.tile([1, 512], f32)
        nc.sync.dma_start(bv_f, bqkv_d[None, 1024:1536])
        bv_sb = const.tile([1, 512], bf16)
        nc.vector.tensor_copy(bv_sb, bv_f)

        # ---- persistent tensors ----
        xT = big.tile([128, NCB, T], bf16, name="xT")
        w_all = big.tile([128, NCB, 1536], bf16, name="w_all")
        wp_sb = big.tile([128, 4, 1024], bf16, name="wp_sb")
        # kdiag mode: qkT is only a DMA staging buffer -> per-segment pool
        qkT = (None if 'kdiag' in opts
               else big.tile([128, 8, T], bf16, name="qkT"))
        v_sb = big.tile([128, H_LOC, NTB, 65], bf16, name="v_sb")
        yt = big.tile([128, 4, T], bf16, name="yt")

        kdiag = qT2 = None
        if 'kdiag' in opts:
            # K=128 attention: block-diagonal K stationaries (one [128,128]
            # tile per head x key-block; off-diagonal quadrants stay zero)
            # and partition-replicated Q, both filled by SBUF->SBUF DMA
            kdiag = big.tile([128, H_LOC, NTB, 128], bf16, name="kdiag")
            nc.gpsimd.memset(kdiag, 0.0)

        nc.gpsimd.memset(v_sb[:, :, :, 64:65], 1.0)

        for _rep in range(reps):
            _emit_v2(nc, tc, mybir, AF, f32, bf16,
                     ptp, ytmp, normp, outp, mmps, sps, yps,
                     xt_d, wqkv_d, wp_d, out_d,
                     xT, w_all, wp_sb, qkT, v_sb, yt,
                     tri, ones1, bqk_sb, bv_sb, phases, opts,
                     kdiag=kdiag, qp2=qp2)
            if dbg:
                nc.sync.dma_start(qkT_d[:, :, :], qkT)
                nc.sync.dma_start(v_d[:, :, :, :], v_sb)
                nc.sync.dma_start(yt_d[:, :, :], yt)

    try:
        nc.compile()
    finally:
        for k, v in _patch.items():
            setattr(hw_specs.TRN2Spec, k, v)
    return nc


def _emit_v2(nc, tc, mybir, AF, f32, bf16,
             ptp, ytmp, normp, outp, mmps, sps, yps,
             xt_d, wqkv_d, wp_d, out_d,
             xT, w_all, wp_sb, qkT, v_sb, yt,
             tri, ones1, bqk_sb, bv_sb, phases, opts,
             kdiag=None, qp2=None):
    use_kd = 'kdiag' in opts
    seg_qT2 = {}
    seg_qk = {}

    def qk_stage(t0):
        if use_kd:
            if t0 not in seg_qk:
                seg_qk[t0] = qp2.tile([128, 8, 512], bf16, name="qks",
                                      tag="qks")
            return seg_qk[t0], 0
        return qkT, t0
    wqk = w_all[:, :, 0:1024]
    wv = w_all[:, :, 1024:1536]
    wqkv_v = wqkv_d.rearrange("(cb p) m -> p cb m", p=128)
    xt_v = xt_d.rearrange("(cb p) t -> p cb t", p=128)

    # ---- upfront DMAs, in first-use order; DMA engines run ahead ----
    nc.sync.dma_start(xT[:, :, 0:512], xt_v[:, :, 0:512])
    nc.sync.dma_start(wv, wqkv_v[:, :, 1024:1536])
    nc.sync.dma_start(wqk, wqkv_v[:, :, 0:1024])
    for ts in range(1, NSEG):
        nc.sync.dma_start(xT[:, :, ts * 512:(ts + 1) * 512],
                          xt_v[:, :, ts * 512:(ts + 1) * 512])
    nc.sync.dma_start(wp_sb, wp_d.rearrange("(pb p) c -> p pb c", p=128))

    def emit_v(tb):
        vp = mmps.tile([128, 512], f32, name="vp", tag="mm")
        nobias = 'nobias' in opts
        for cb in range(NCB):
            nc.tensor.matmul(
                vp, xT[:, cb, tb * 128:(tb + 1) * 128],
                wv[:, cb, :], start=(cb == 0), stop=(nobias and cb == NCB - 1))
        if not nobias:
            # bias via K=1 matmul: ones1^T @ bv adds bv to every row
            nc.tensor.matmul(vp, ones1, bv_sb, start=False, stop=True)
        nc.vector.tensor_copy(
            v_sb[:, :, tb, 0:64],
            vp.rearrange("p (h d) -> p h d", h=H_LOC))

    def emit_qk(mb, t0):
        qp = mmps.tile([128, 512], f32, name="qp", tag="mm")
        for cb in range(NCB):
            nc.tensor.matmul(
                qp, wqk[:, cb, mb * 128:(mb + 1) * 128],
                xT[:, cb, t0:t0 + 512],
                start=(cb == 0), stop=(cb == NCB - 1))
        stg, off = qk_stage(t0)
        if 'nobias' in opts:
            nc.vector.tensor_copy(stg[:, mb, off:off + 512], qp)
        else:
            nc.vector.tensor_scalar_add(
                stg[:, mb, off:off + 512], qp, bqk_sb[:, mb:mb + 1])

    def emit_repl(ts, pr):
        # build the K=128 attention operands for segment ts, head pair pr:
        # qT2[*, h, :] = q_h replicated on both partition halves;
        # kdiag[*, h, kb, :] = block-diag([k_h 64-key chunk, next chunk]).
        # Per-pair small DMAs spread through the pipeline measure faster
        # than per-segment batched ones; the Act HWDGE queue keeps them off
        # the SP queue that carries the bulk input loads.
        t0 = ts * 512
        if ts not in seg_qT2:
            seg_qT2[ts] = qp2.tile([128, H_LOC, 512], bf16, name="qT2s",
                                   tag="qT2s")
        qT2s = seg_qT2[ts]
        stg, off = qk_stage(t0)
        dma_eng = nc.sync if 'replsp' in opts else nc.scalar
        for i in range(2):
            h = 2 * pr + i
            hs = 64 * i
            qsrc = stg[hs:hs + 64, 2 * pr, off:off + 512]
            dma_eng.dma_start(qT2s[0:64, h, :], qsrc)
            dma_eng.dma_start(qT2s[64:128, h, :], qsrc)
            kv = stg[hs:hs + 64, 2 * pr + 1, off:off + 512].rearrange(
                "p (j f) -> p j f", j=4)
            dma_eng.dma_start(kdiag[0:64, h, 4 * ts:4 * ts + 4, 0:64],
                              kv[:, :, 0:64])
            dma_eng.dma_start(kdiag[64:128, h, 4 * ts:4 * ts + 4, 64:128],
                              kv[:, :, 64:128])

    def emit_attention(ts, pr):
        q0 = ts * 512
        nkb = 4 * (ts + 1)
        if not use_kd:
            qT0 = qkT[0:64, 2 * pr, :]
            kT0 = qkT[0:64, 2 * pr + 1, :]
            qT1 = qkT[64:128, 2 * pr, :]
            kT1 = qkT[64:128, 2 * pr + 1, :]
        qT2s = seg_qT2.get(ts)
        y0 = yps.tile([65, 512], f32, name="y0", tag="y")
        y1 = yps.tile([65, 512], f32, name="y1", tag="y")

        def emit_av(kb, pt, qlo):
            off = qlo - q0
            for i, y_ps in ((0, y0), (1, y1)):
                nc.tensor.matmul(
                    y_ps[:, off:512], v_sb[:, 2 * pr + i, kb, :],
                    pt[:, i, 0:512 - off],
                    start=(kb == 0), stop=(kb == nkb - 1),
                    skip_group_check=True)

        pending = None
        for kb in range(nkb):
            qlo = max(q0, kb * 128)
            qlen = q0 + 512 - qlo
            sp = sps.tile([128, 2, 512], f32, name="sp")
            if use_kd:
                # K=128 S^T via block-diagonal K stationary + replicated Q
                off = qlo - q0
                nc.tensor.matmul(sp[:, 0, 0:qlen],
                                 kdiag[:, 2 * pr, kb, :],
                                 qT2s[:, 2 * pr, off:off + qlen],
                                 start=True, stop=True)
                nc.tensor.matmul(sp[:, 1, 0:qlen],
                                 kdiag[:, 2 * pr + 1, kb, :],
                                 qT2s[:, 2 * pr + 1, off:off + qlen],
                                 start=True, stop=True)
            else:
                # paired S^T: disjoint PE row groups -> concurrent on HW
                nc.tensor.matmul(sp[:, 0, 0:qlen],
                                 kT0[:, kb * 128:(kb + 1) * 128],
                                 qT0[:, qlo:qlo + qlen],
                                 start=True, stop=True)
                nc.tensor.matmul(sp[:, 1, 0:qlen],
                                 kT1[:, kb * 128:(kb + 1) * 128],
                                 qT1[:, qlo:qlo + qlen],
                                 start=True, stop=True)
            pt = ptp.tile([128, 2, 512], bf16, name="pt")
            if 'flatexp' in opts and qlen == 512:
                nc.scalar.activation(
                    pt.rearrange("p i q -> p (i q)"),
                    sp.rearrange("p i q -> p (i q)"),
                    AF.Exp, scale=0.125)
            else:
                nc.scalar.activation(pt[:, :, 0:qlen], sp[:, :, 0:qlen],
                                     AF.Exp, scale=0.125)
            if kb * 128 >= q0:
                # diagonal block: zero the strictly-upper part
                nc.gpsimd.tensor_mul(pt[:, 0, 0:128], pt[:, 0, 0:128], tri)
                nc.gpsimd.tensor_mul(pt[:, 1, 0:128], pt[:, 1, 0:128], tri)
            if 'dbg' in opts and ts == 0 and pr == 0 and kb == 0:
                stg = ytmp.tile([128, 2, 512], f32, name="spstg",
                                tag="dbgst")
                nc.vector.tensor_copy(stg, sp[:, :, :])
                nc.sync.dma_start(_dbg_tensors["sp"][:, :, :], stg)
                nc.sync.dma_start(_dbg_tensors["pt"][:, :, :], pt[:, :, :])
            if pending is not None:
                emit_av(*pending)
            pending = (kb, pt, qlo)
        emit_av(*pending)

        if 'pairnorm' in opts:
            # batched normalization for the head pair: evacuate both y
            # PSUMs into one [128,512] tile, one recip over both sums
            # rows, one 128-partition multiply
            ytm2 = ytmp.tile([128, 512], f32, name="ytm2", tag="ytm2")
            sums2 = normp.tile([1, 2, 512], f32, name="sums2", tag="sums2")
            nc.vector.tensor_copy(ytm2[0:64, :], y0[0:64, :])
            nc.vector.tensor_copy(sums2[:, 0, :], y0[64:65, :])
            nc.vector.tensor_copy(ytm2[64:128, :], y1[0:64, :])
            nc.vector.tensor_copy(sums2[:, 1, :], y1[64:65, :])
            recip2 = normp.tile([1, 2, 512], f32, name="recip2",
                                tag="recip2")
            nc.vector.reciprocal_approx_fast(
                recip2.rearrange("p i q -> p (i q)"),
                sums2.rearrange("p i q -> p (i q)"))
            bc2 = normp.tile([128, 512], f32, name="bc2", tag="bc2")
            nc.gpsimd.partition_broadcast(bc2[0:64, :], recip2[:, 0, :])
            nc.gpsimd.partition_broadcast(bc2[64:128, :], recip2[:, 1, :])
            nc.vector.tensor_mul(yt[:, pr, q0:q0 + 512], ytm2, bc2)
            return
        # deferred normalization: one copy evacuates PSUM; the sums row is
        # re-staged to a base-0 tile (custom-DVE recip needs base 0), the
        # rest runs from SBUF off the critical path
        for i, (y_ps, po) in enumerate(((y0, 0), (y1, 64))):
            if 'sttnorm' in opts:
                # fused path: y stays in PSUM until bc is ready
                sums = normp.tile([1, 512], f32, name="sums")
                nc.vector.tensor_copy(sums, y_ps[64:65, :])
                recip = normp.tile([1, 512], f32, name="recip")
                nc.vector.reciprocal_approx_fast(recip, sums)
                bc = normp.tile([64, 512], f32, name="bc")
                nc.gpsimd.partition_broadcast(bc, recip)
                nc.vector.scalar_tensor_tensor(
                    out=yt[po:po + 64, pr, q0:q0 + 512], in0=y_ps[0:64, :],
                    scalar=1.0, in1=bc,
                    op0=mybir.AluOpType.mult, op1=mybir.AluOpType.mult)
                continue
            ytm = ytmp.tile([65, 512], f32, name="ytm")
            nc.vector.tensor_copy(ytm, y_ps)
            sums = normp.tile([1, 512], f32, name="sums")
            nc.vector.tensor_copy(sums, ytm[64:65, :])
            recip = normp.tile([1, 512], f32, name="recip")
            nc.vector.reciprocal_approx_fast(recip, sums)
            bc = normp.tile([64, 512], f32, name="bc")
            nc.gpsimd.partition_broadcast(bc, recip)
            if 'dbg' in opts and ts == 0 and pr == 0 and i == 0:
                nc.sync.dma_start(_dbg_tensors["ytm"][:, :], ytm)
                nc.sync.dma_start(_dbg_tensors["recip"][:, :], recip)
                nc.sync.dma_start(_dbg_tensors["bc"][:, :], bc)
            nc.vector.tensor_mul(
                yt[po:po + 64, pr, q0:q0 + 512], ytm[0:64, :], bc)

    def emit_proj(tb):
        o_sb = outp.tile([128, 1024], bf16, name="o_sb")
        for ns in range(2):
            pp = mmps.tile([128, 512], f32, name="pp", tag="mm")
            for p in range(4):
                nc.tensor.matmul(
                    pp, yt[:, p, tb * 128:(tb + 1) * 128],
                    wp_sb[:, p, ns * 512:(ns + 1) * 512],
                    start=(p == 0), stop=(p == 3))
            nc.vector.tensor_copy(o_sb[:, ns * 512:(ns + 1) * 512], pp)
        if 'nostore' not in opts:
            if 'storesync' in opts:
                nc.sync.dma_start(out_d[tb * 128:(tb + 1) * 128, :], o_sb)
            else:
                nc.gpsimd.dma_start(out_d[tb * 128:(tb + 1) * 128, :], o_sb)

    # ---- segment 0 QKV: V first, then per-pr QK so attention(0, pr)
    # can start as soon as its own Q/K blocks land ----
    eager0 = 'eager0' in opts
    if 'B' in phases:
        for tb in range(4):
            emit_v(tb)
        if not eager0:
            if use_kd:
                for pr in range(4):
                    emit_qk(2 * pr, 0)
                    emit_qk(2 * pr + 1, 0)
                    emit_repl(0, pr)
            else:
                for mb in range(8):
                    emit_qk(mb, 0)

    # ---- steady state: attention(ts) with next-segment QKV and
    # previous-segment proj pieces interleaved to fill PE gaps ----
    projend = 'projil' not in opts
    for ts in range(NSEG):
        for pr in range(4):
            if ts == 0 and eager0 and 'B' in phases:
                emit_qk(2 * pr, 0)
                emit_qk(2 * pr + 1, 0)
                if use_kd:
                    emit_repl(0, pr)
            if 'C' in phases:
                emit_attention(ts, pr)
            if ts < NSEG - 1 and 'B' in phases:
                emit_v(4 * (ts + 1) + pr)
                emit_qk(2 * pr, (ts + 1) * 512)
                emit_qk(2 * pr + 1, (ts + 1) * 512)
                if use_kd:
                    emit_repl(ts + 1, pr)
            if not projend and ts > 0 and 'D' in phases:
                emit_proj(4 * (ts - 1) + pr)
        if projend and 'D' in phases:
            for tb in range(4 * ts, 4 * ts + 4):
                emit_proj(tb)
    if not projend and 'D' in phases:
        for tb in range(4 * (NSEG - 1), 4 * NSEG):
            emit_proj(tb)


def _shard_inputs(x, w_attn, b_attn, w_proj):
    """Build per-core input maps (pair-packed q/k layouts; see module doc)."""
    wq = w_attn[:, 0:C].reshape(C, N_HEAD, D)
    wk = w_attn[:, C:2 * C].reshape(C, N_HEAD, D)
    wv = w_attn[:, 2 * C:3 * C].reshape(C, N_HEAD, D)
    bq = b_attn[0:C].reshape(N_HEAD, D)
    bk = b_attn[C:2 * C].reshape(N_HEAD, D)
    bv = b_attn[2 * C:3 * C].reshape(N_HEAD, D)

    xt_by_batch = [
        np.ascontiguousarray(x[b].T).astype(ml_dtypes.bfloat16)
        for b in range(B)
    ]

    in_maps = []
    for core in range(N_CORES):
        b, g = core // 2, core % 2
        h0 = g * H_LOC
        qk_blocks, bqk_parts = [], []
        for p in range(4):
            hA, hB = h0 + 2 * p, h0 + 2 * p + 1
            qk_blocks.append(np.concatenate([wq[:, hA], wq[:, hB]], axis=1))
            qk_blocks.append(np.concatenate([wk[:, hA], wk[:, hB]], axis=1))
            bqk_parts.append(np.concatenate([bq[hA], bq[hB]]))
            bqk_parts.append(np.concatenate([bk[hA], bk[hB]]))
        wqkv = np.concatenate(
            qk_blocks + [wv[:, h0:h0 + H_LOC].reshape(C, H_LOC * D)], axis=1)
        bqkv = np.concatenate(
            bqk_parts + [bv[h0:h0 + H_LOC].reshape(H_LOC * D)])
        wproj = w_proj.reshape(N_HEAD, D, C)[h0:h0 + H_LOC].reshape(
            H_LOC * D, C)
        in_maps.append({
            "xt": xt_by_batch[b],
            "wqkv": np.ascontiguousarray(wqkv).astype(ml_dtypes.bfloat16),
            "bqkv": np.ascontiguousarray(bqkv, dtype=np.float32),
            "wproj": np.ascontiguousarray(wproj).astype(ml_dtypes.bfloat16),
        })
    return in_maps


def kernel(x, w_attn, b_attn, w_proj, b_proj):
    global last_exec_ns
    from concourse.bass_utils import run_bass_kernel_spmd

    x = np.asarray(x, dtype=np.float32)
    w_attn = np.asarray(w_attn, dtype=np.float32)
    b_attn = np.asarray(b_attn, dtype=np.float32)
    w_proj = np.asarray(w_proj, dtype=np.float32)
    b_proj = np.asarray(b_proj, dtype=np.float32)

    kenv = os.environ.get("KOPTS")
    if kenv is not None:
        kopts = tuple(o for o in kenv.split(",") if o)
    else:
        # default: K=128 block-diagonal attention (fastest measured variant)
        kopts = ("kdiag",)
    if _cache.get("opts") != kopts:
        _cache["nc"] = _build_program(opts=kopts)
        _cache["opts"] = kopts
    nc = _cache["nc"]

    in_maps = _shard_inputs(x, w_attn, b_attn, w_proj)
    trace = os.environ.get("KERNEL_TRACE", "0") == "1"
    if trace:
        try:
            import antenv.axon_hooks  # noqa: F401
        except ImportError:
            trace = False
    res = run_bass_kernel_spmd(nc, in_maps, core_ids=list(range(N_CORES)),
                               trace=trace)
    last_exec_ns = res.exec_time_ns

    out = np.empty((B, T, C), dtype=np.float32)
    for b in range(B):
        out[b] = (res.results[2 * b]["out"].astype(np.float32)
                  + res.results[2 * b + 1]["out"].astype(np.float32)
                  + b_proj[None, :])
    return out



# revision 26
# speedup vs baseline: 1.1800x; 1.1026x over previous
"""Trainium2 Bass kernel: GPT-style causal self-attention block.

Computes, for x[B=4, T=2048, C=1024], 16 heads x 64 dims:
    qkv = x @ w_attn + b_attn ; causal softmax attention ; y @ w_proj + b_proj

Sharding (8 cores): data-parallel over B (4) x tensor-parallel over head
groups (2 groups of 8 heads, Megatron style).  Each core:
  - receives x^T (host-transposed) and its slice of the weights,
  - computes Q/K/V projections and runs causal attention per head pair,
  - applies its row-slice of w_proj (row-parallel) producing a partial
    [T, C] output.  Host sums the two partials per batch and adds b_proj.

Attention ('kdiag', the default): measured HW runs K=64 matmuls ~2.7x
slower per streamed column than K=128, so S^T = K^T Q uses full-K=128
matmuls built from
  - kdiag: one [128,128] block-diagonal stationary per (head, 128-key
    block) — k-chunk d-vectors on each partition half, zero quadrants
    memset once — and
  - qT2: q_h replicated on both partition halves (SBUF->SBUF DMA on the
    Act HWDGE queue, off the SP queue carrying bulk input loads).
With 'krep' (used when b_attn == 0): the K projection instead uses
per-head stationaries with duplicated columns, so kp comes out
partition-replicated and kdiag fills via two lane-aligned DVE copies --
no DMAs (each HWDGE trigger costs ~650ns on a shared single-slot
frontend and holds the issuing sequencer, which serialized with the exp
stream).  Only the 4-per-pair qT2 DMAs remain.

One ScalarE exp instruction covers both heads' S tiles; AV matmuls carry
a ones-column so softmax denominators fall out of the same accumulation;
normalization is deferred off the PSUM critical path (DVE copy evacuates
y+sums, then recip/broadcast/scale from SBUF).

The per-512-token-segment loop interleaves QKV -> attention -> proj so
PE work fills the gaps in the ScalarE-bound softmax stream.
"""

import os
import ml_dtypes
import numpy as np

B, T, C = 4, 2048, 1024
N_HEAD = 16
D = 64  # head dim
H_LOC = 8  # heads per core
N_CORES = 8

NTB = T // 128   # 16 token blocks
NCB = C // 128   # 8 contraction blocks
NSEG = T // 512  # 4 token segments
QQ = 512         # attention q-tile width

_cache = {}
_dbg_tensors = {}

last_exec_ns = None


def _build_program(reps=1, phases='ABCD', opts=()):
    from contextlib import ExitStack

    import concourse.bass as bass
    import concourse.mybir as mybir
    import concourse.tile as tile
    from concourse import bacc

    f32 = mybir.dt.float32
    bf16 = mybir.dt.bfloat16
    AF = mybir.ActivationFunctionType

    import concourse.hw_specs as hw_specs
    _patch = {}
    if 'pe32' in opts:
        # scheduling-only hint: measured HW bf16 matmul throughput is
        # ~3.24 G cols/s in steady state (mm512x8 microbench: 158 ns per
        # 512-col K=128 matmul); restored before return
        _patch = {"PE_CYCLE": hw_specs.TRN2Spec.PE_CYCLE,
                  "PE_CYCLE_PSTATE_MID": hw_specs.TRN2Spec.PE_CYCLE_PSTATE_MID}
        hw_specs.TRN2Spec.PE_CYCLE = 1e9 / 3.24e9
        hw_specs.TRN2Spec.PE_CYCLE_PSTATE_MID = 1e9 / 3.24e9
    elif 'fastpe' in opts:
        # scheduling-only hint: match the cost model to measured HW matmul
        # throughput (bf16 ~4x the default model) while building; restored
        # before return so no global state leaks
        _patch = {"PE_CYCLE": hw_specs.TRN2Spec.PE_CYCLE,
                  "PE_CYCLE_PSTATE_MID": hw_specs.TRN2Spec.PE_CYCLE_PSTATE_MID}
        hw_specs.TRN2Spec.PE_CYCLE = 1e9 / 9.6e9
        hw_specs.TRN2Spec.PE_CYCLE_PSTATE_MID = 1e9 / 4.8e9

    nc = bacc.Bacc("TRN2", target_bir_lowering=False, debug=False,
                   num_devices=N_CORES)

    xt_d = nc.dram_tensor("xt", [C, T], bf16, kind="ExternalInput")
    wqkv_w = 2048 if 'krep' in opts else 1536
    wqkv_d = nc.dram_tensor("wqkv", [C, wqkv_w], bf16, kind="ExternalInput")
    bqkv_d = nc.dram_tensor("bqkv", [1536], f32, kind="ExternalInput")
    wp_d = nc.dram_tensor("wproj", [512, C], bf16, kind="ExternalInput")
    out_d = nc.dram_tensor("out", [T, C], bf16, kind="ExternalOutput")
    dbg = 'dbg' in opts
    if dbg:
        qkT_d = nc.dram_tensor("qkT_dbg", [128, 8, T], bf16,
                               kind="ExternalOutput")
        v_d = nc.dram_tensor("v_dbg", [128, H_LOC, NTB, 65], bf16,
                             kind="ExternalOutput")
        yt_d = nc.dram_tensor("yt_dbg", [128, 4, T], bf16,
                              kind="ExternalOutput")
        sp_d = nc.dram_tensor("sp_dbg", [128, 2, 512], f32,
                              kind="ExternalOutput")
        pt_d = nc.dram_tensor("pt_dbg", [128, 2, 512], bf16,
                              kind="ExternalOutput")
        ytm_d = nc.dram_tensor("ytm_dbg", [65, 512], f32,
                               kind="ExternalOutput")
        recip_d = nc.dram_tensor("recip_dbg", [1, 512], f32,
                                 kind="ExternalOutput")
        bc_d = nc.dram_tensor("bc_dbg", [64, 512], f32,
                              kind="ExternalOutput")
        _dbg_tensors["sp"] = sp_d
        _dbg_tensors["pt"] = pt_d
        _dbg_tensors["ytm"] = ytm_d
        _dbg_tensors["recip"] = recip_d
        _dbg_tensors["bc"] = bc_d

    with ExitStack() as ctx:
        tc = ctx.enter_context(tile.TileContext(nc))

        const = ctx.enter_context(tc.tile_pool(name="const", bufs=1))
        big = ctx.enter_context(tc.tile_pool(name="big", bufs=1))
        qp2 = ctx.enter_context(tc.tile_pool(name="qp2", bufs=2))
        ptp = ctx.enter_context(tc.tile_pool(
            name="ptp", bufs=(4 if 'ptp4' in opts else 3)))
        ytmp = ctx.enter_context(tc.tile_pool(name="ytmp", bufs=3))
        normp = ctx.enter_context(tc.tile_pool(name="normp", bufs=3))
        outp = ctx.enter_context(tc.tile_pool(name="outp", bufs=3))
        y3 = 'y3' in opts
        mmps = ctx.enter_context(tc.tile_pool(name="mmps",
                                              bufs=(1 if y3 else 2),
                                              space="PSUM"))
        sps = ctx.enter_context(tc.tile_pool(name="sps", bufs=2,
                                             space="PSUM"))
        yps = ctx.enter_context(tc.tile_pool(name="yps",
                                             bufs=(3 if y3 else 2),
                                             space="PSUM"))

        # ---- constants ----
        # tri[k, q] = 1.0 where q >= k else 0 (multiplicative causal mask
        # for the diagonal 128x128 block of an S^T tile)
        tri = const.tile([128, 128], bf16)
        nc.gpsimd.memset(tri, 1.0)
        nc.gpsimd.affine_select(
            out=tri, in_=tri, compare_op=mybir.AluOpType.is_ge,
            fill=0.0, base=0, pattern=[[1, 128]], channel_multiplier=-1,
        )
        ones1 = const.tile([1, 128], bf16)
        nc.gpsimd.memset(ones1, 1.0)

        # qk bias, one column per m-block: bqk_sb[p, mb] = bqkv[mb*128 + p]
        bqk_sb = const.tile([128, 8], f32)
        nc.sync.dma_start(bqk_sb,
                          bqkv_d[0:1024].rearrange("(mb p) -> p mb", p=128))
        bv_f = const.tile([1, 512], f32)
        nc.sync.dma_start(bv_f, bqkv_d[None, 1024:1536])
        bv_sb = const.tile([1, 512], bf16)
        nc.vector.tensor_copy(bv_sb, bv_f)

        # ---- persistent tensors ----
        xT = big.tile([128, NCB, T], bf16, name="xT")
        w_all = big.tile([128, NCB, wqkv_w], bf16, name="w_all")
        wp_sb = big.tile([128, 4, 1024], bf16, name="wp_sb")
        # kdiag mode: qkT is only a DMA staging buffer -> per-segment pool
        qkT = (None if 'kdiag' in opts
               else big.tile([128, 8, T], bf16, name="qkT"))
        v_sb = big.tile([128, H_LOC, NTB, 65], bf16, name="v_sb")
        yt = big.tile([128, 4, T], bf16, name="yt")

        kdiag = qT2 = None
        if 'kdiag' in opts:
            # K=128 attention: block-diagonal K stationaries (one [128,128]
            # tile per head x key-block; off-diagonal quadrants stay zero)
            # and partition-replicated Q, both filled by SBUF->SBUF DMA
            kdiag = big.tile([128, H_LOC, NTB, 128], bf16, name="kdiag")
            nc.gpsimd.memset(kdiag, 0.0)

        nc.gpsimd.memset(v_sb[:, :, :, 64:65], 1.0)

        for _rep in range(reps):
            _emit_v2(nc, tc, mybir, AF, f32, bf16,
                     ptp, ytmp, normp, outp, mmps, sps, yps,
                     xt_d, wqkv_d, wp_d, out_d,
                     xT, w_all, wp_sb, qkT, v_sb, yt,
                     tri, ones1, bqk_sb, bv_sb, phases, opts,
                     kdiag=kdiag, qp2=qp2)
            if dbg:
                nc.sync.dma_start(qkT_d[:, :, :], qkT)
                nc.sync.dma_start(v_d[:, :, :, :], v_sb)
                nc.sync.dma_start(yt_d[:, :, :], yt)

    try:
        nc.compile()
    finally:
        for k, v in _patch.items():
            setattr(hw_specs.TRN2Spec, k, v)
    return nc


def _emit_v2(nc, tc, mybir, AF, f32, bf16,
             ptp, ytmp, normp, outp, mmps, sps, yps,
             xt_d, wqkv_d, wp_d, out_d,
             xT, w_all, wp_sb, qkT, v_sb, yt,
             tri, ones1, bqk_sb, bv_sb, phases, opts,
             kdiag=None, qp2=None):
    use_kd = 'kdiag' in opts
    krep = 'krep' in opts
    assert not krep or (use_kd and 'nobias' in opts), \
        "krep requires kdiag+nobias"
    seg_qT2 = {}
    seg_qk = {}

    def qk_stage(t0):
        if use_kd:
            if t0 not in seg_qk:
                seg_qk[t0] = qp2.tile([128, 4 if krep else 8, 512], bf16,
                                      name="qks", tag="qks")
            return seg_qk[t0], 0
        return qkT, t0
    if krep:
        # [q-pair0..3 | k-head0..7 (columns duplicated) | v]
        wqk = w_all[:, :, 0:512]
        w_k = w_all[:, :, 512:1536]
        wv = w_all[:, :, 1536:2048]
    else:
        wqk = w_all[:, :, 0:1024]
        w_k = None
        wv = w_all[:, :, 1024:1536]
    wqkv_v = wqkv_d.rearrange("(cb p) m -> p cb m", p=128)
    xt_v = xt_d.rearrange("(cb p) t -> p cb t", p=128)
    # ---- upfront DMAs, in first-use order; DMA engines run ahead ----
    nc.sync.dma_start(xT[:, :, 0:512], xt_v[:, :, 0:512])
    if krep:
        nc.sync.dma_start(wv, wqkv_v[:, :, 1536:2048])
        nc.sync.dma_start(wqk, wqkv_v[:, :, 0:512])
        nc.sync.dma_start(w_k, wqkv_v[:, :, 512:1536])
    else:
        nc.sync.dma_start(wv, wqkv_v[:, :, 1024:1536])
        nc.sync.dma_start(wqk, wqkv_v[:, :, 0:1024])
    for ts in range(1, NSEG):
        nc.sync.dma_start(xT[:, :, ts * 512:(ts + 1) * 512],
                          xt_v[:, :, ts * 512:(ts + 1) * 512])
    nc.sync.dma_start(wp_sb, wp_d.rearrange("(pb p) c -> p pb c", p=128))

    def emit_v(tb):
        vp = mmps.tile([128, 512], f32, name="vp", tag="mm")
        nobias = 'nobias' in opts
        for cb in range(NCB):
            nc.tensor.matmul(
                vp, xT[:, cb, tb * 128:(tb + 1) * 128],
                wv[:, cb, :], start=(cb == 0), stop=(nobias and cb == NCB - 1))
        if not nobias:
            # bias via K=1 matmul: ones1^T @ bv adds bv to every row
            nc.tensor.matmul(vp, ones1, bv_sb, start=False, stop=True)
        nc.vector.tensor_copy(
            v_sb[:, :, tb, 0:64],
            vp.rearrange("p (h d) -> p h d", h=H_LOC))

    def emit_qk(mb, t0):
        qp = mmps.tile([128, 512], f32, name="qp", tag="mm")
        for cb in range(NCB):
            nc.tensor.matmul(
                qp, wqk[:, cb, mb * 128:(mb + 1) * 128],
                xT[:, cb, t0:t0 + 512],
                start=(cb == 0), stop=(cb == NCB - 1))
        stg, off = qk_stage(t0)
        if 'nobias' in opts:
            nc.vector.tensor_copy(stg[:, mb, off:off + 512], qp)
        else:
            nc.vector.tensor_scalar_add(
                stg[:, mb, off:off + 512], qp, bqk_sb[:, mb:mb + 1])

    def emit_kh(h, t0):
        # k projection for one head with duplicated stationary columns ->
        # kp comes out replicated on both partition halves; the block-diag
        # kdiag tiles then fill with two lane-aligned DVE copies (no DMA)
        kp = mmps.tile([128, 512], f32, name="kp", tag="mm")
        for cb in range(NCB):
            nc.tensor.matmul(
                kp, w_k[:, cb, h * 128:(h + 1) * 128],
                xT[:, cb, t0:t0 + 512],
                start=(cb == 0), stop=(cb == NCB - 1))
        ts = t0 // 512
        top = kp[0:64, :].rearrange("p (j c f) -> p c j f", c=2, f=64)
        bot = kp[64:128, :].rearrange("p (j c f) -> p c j f", c=2, f=64)
        nc.vector.tensor_copy(
            kdiag[0:64, h, 4 * ts:4 * ts + 4, 0:64], top[:, 0])
        nc.vector.tensor_copy(
            kdiag[64:128, h, 4 * ts:4 * ts + 4, 64:128], bot[:, 1])

    def emit_repl(ts, pr):
        # build the K=128 attention operands for segment ts, head pair pr:
        # qT2[*, h, :] = q_h replicated on both partition halves;
        # kdiag[*, h, kb, :] = block-diag([k_h 64-key chunk, next chunk]).
        # Per-pair small DMAs spread through the pipeline measure faster
        # than per-segment batched ones; the Act HWDGE queue keeps them off
        # the SP queue that carries the bulk input loads.
        t0 = ts * 512
        if ts not in seg_qT2:
            seg_qT2[ts] = qp2.tile([128, H_LOC, 512], bf16, name="qT2s",
                                   tag="qT2s")
        qT2s = seg_qT2[ts]
        stg, off = qk_stage(t0)
        dma_eng = nc.sync if 'replsp' in opts else nc.scalar
        qcol = pr if krep else 2 * pr
        for i in range(2):
            h = 2 * pr + i
            hs = 64 * i
            qsrc = stg[hs:hs + 64, qcol, off:off + 512]
            dma_eng.dma_start(qT2s[0:64, h, :], qsrc)
            dma_eng.dma_start(qT2s[64:128, h, :], qsrc)
            if not krep:
                kv = stg[hs:hs + 64, 2 * pr + 1, off:off + 512].rearrange(
                    "p (j f) -> p j f", j=4)
                dma_eng.dma_start(kdiag[0:64, h, 4 * ts:4 * ts + 4, 0:64],
                                  kv[:, :, 0:64])
                dma_eng.dma_start(kdiag[64:128, h, 4 * ts:4 * ts + 4, 64:128],
                                  kv[:, :, 64:128])

    def emit_attention(ts, pr):
        q0 = ts * 512
        nkb = 4 * (ts + 1)
        if not use_kd:
            qT0 = qkT[0:64, 2 * pr, :]
            kT0 = qkT[0:64, 2 * pr + 1, :]
            qT1 = qkT[64:128, 2 * pr, :]
            kT1 = qkT[64:128, 2 * pr + 1, :]
        qT2s = seg_qT2.get(ts)
        y0 = yps.tile([65, 512], f32, name="y0", tag="y")
        y1 = yps.tile([65, 512], f32, name="y1", tag="y")

        def emit_av(kb, pt, qlo):
            off = qlo - q0
            for i, y_ps in ((0, y0), (1, y1)):
                nc.tensor.matmul(
                    y_ps[:, off:512], v_sb[:, 2 * pr + i, kb, :],
                    pt[:, i, 0:512 - off],
                    start=(kb == 0), stop=(kb == nkb - 1),
                    skip_group_check=True)

        pending = None
        for kb in range(nkb):
            qlo = max(q0, kb * 128)
            qlen = q0 + 512 - qlo
            sp = sps.tile([128, 2, 512], f32, name="sp")
            if use_kd:
                # K=128 S^T via block-diagonal K stationary + replicated Q
                off = qlo - q0
                nc.tensor.matmul(sp[:, 0, 0:qlen],
                                 kdiag[:, 2 * pr, kb, :],
                                 qT2s[:, 2 * pr, off:off + qlen],
                                 start=True, stop=True)
                nc.tensor.matmul(sp[:, 1, 0:qlen],
                                 kdiag[:, 2 * pr + 1, kb, :],
                                 qT2s[:, 2 * pr + 1, off:off + qlen],
                                 start=True, stop=True)
            else:
                # paired S^T: disjoint PE row groups -> concurrent on HW
                nc.tensor.matmul(sp[:, 0, 0:qlen],
                                 kT0[:, kb * 128:(kb + 1) * 128],
                                 qT0[:, qlo:qlo + qlen],
                                 start=True, stop=True)
                nc.tensor.matmul(sp[:, 1, 0:qlen],
                                 kT1[:, kb * 128:(kb + 1) * 128],
                                 qT1[:, qlo:qlo + qlen],
                                 start=True, stop=True)
            pt = ptp.tile([128, 2, 512], bf16, name="pt")
            if 'flatexp' in opts and qlen == 512:
                nc.scalar.activation(
                    pt.rearrange("p i q -> p (i q)"),
                    sp.rearrange("p i q -> p (i q)"),
                    AF.Exp, scale=0.125)
            else:
                nc.scalar.activation(pt[:, :, 0:qlen], sp[:, :, 0:qlen],
                                     AF.Exp, scale=0.125)
            if kb * 128 >= q0:
                # diagonal block: zero the strictly-upper part
                nc.gpsimd.tensor_mul(pt[:, 0, 0:128], pt[:, 0, 0:128], tri)
                nc.gpsimd.tensor_mul(pt[:, 1, 0:128], pt[:, 1, 0:128], tri)
            if 'dbg' in opts and ts == 0 and pr == 0 and kb == 0:
                stg = ytmp.tile([128, 2, 512], f32, name="spstg",
                                tag="dbgst")
                nc.vector.tensor_copy(stg, sp[:, :, :])
                nc.sync.dma_start(_dbg_tensors["sp"][:, :, :], stg)
                nc.sync.dma_start(_dbg_tensors["pt"][:, :, :], pt[:, :, :])
            if pending is not None:
                emit_av(*pending)
            pending = (kb, pt, qlo)
        emit_av(*pending)

        if 'pairnorm' in opts:
            # batched normalization for the head pair: evacuate both y
            # PSUMs into one [128,512] tile, one recip over both sums
            # rows, one 128-partition multiply
            ytm2 = ytmp.tile([128, 512], f32, name="ytm2", tag="ytm2")
            sums2 = normp.tile([1, 2, 512], f32, name="sums2", tag="sums2")
            nc.vector.tensor_copy(ytm2[0:64, :], y0[0:64, :])
            nc.vector.tensor_copy(sums2[:, 0, :], y0[64:65, :])
            nc.vector.tensor_copy(ytm2[64:128, :], y1[0:64, :])
            nc.vector.tensor_copy(sums2[:, 1, :], y1[64:65, :])
            recip2 = normp.tile([1, 2, 512], f32, name="recip2",
                                tag="recip2")
            nc.vector.reciprocal_approx_fast(
                recip2.rearrange("p i q -> p (i q)"),
                sums2.rearrange("p i q -> p (i q)"))
            bc2 = normp.tile([128, 512], f32, name="bc2", tag="bc2")
            nc.gpsimd.partition_broadcast(bc2[0:64, :], recip2[:, 0, :])
            nc.gpsimd.partition_broadcast(bc2[64:128, :], recip2[:, 1, :])
            nc.vector.tensor_mul(yt[:, pr, q0:q0 + 512], ytm2, bc2)
            return
        # deferred normalization: one copy evacuates PSUM; the sums row is
        # re-staged to a base-0 tile (custom-DVE recip needs base 0), the
        # rest runs from SBUF off the critical path
        for i, (y_ps, po) in enumerate(((y0, 0), (y1, 64))):
            if 'sttnorm' in opts:
                # fused path: y stays in PSUM until bc is ready
                sums = normp.tile([1, 512], f32, name="sums")
                nc.vector.tensor_copy(sums, y_ps[64:65, :])
                recip = normp.tile([1, 512], f32, name="recip")
                nc.vector.reciprocal_approx_fast(recip, sums)
                bc = normp.tile([64, 512], f32, name="bc")
                nc.gpsimd.partition_broadcast(bc, recip)
                nc.vector.scalar_tensor_tensor(
                    out=yt[po:po + 64, pr, q0:q0 + 512], in0=y_ps[0:64, :],
                    scalar=1.0, in1=bc,
                    op0=mybir.AluOpType.mult, op1=mybir.AluOpType.mult)
                continue
            ytm = ytmp.tile([65, 512], f32, name="ytm")
            nc.vector.tensor_copy(ytm, y_ps)
            sums = normp.tile([1, 512], f32, name="sums")
            nc.vector.tensor_copy(sums, ytm[64:65, :])
            recip = normp.tile([1, 512], f32, name="recip")
            nc.vector.reciprocal_approx_fast(recip, sums)
            bc = normp.tile([64, 512], f32, name="bc")
            nc.gpsimd.partition_broadcast(bc, recip)
            if 'dbg' in opts and ts == 0 and pr == 0 and i == 0:
                nc.sync.dma_start(_dbg_tensors["ytm"][:, :], ytm)
                nc.sync.dma_start(_dbg_tensors["recip"][:, :], recip)
                nc.sync.dma_start(_dbg_tensors["bc"][:, :], bc)
            nc.vector.tensor_mul(
                yt[po:po + 64, pr, q0:q0 + 512], ytm[0:64, :], bc)

    def emit_proj(tb):
        o_sb = outp.tile([128, 1024], bf16, name="o_sb")
        for ns in range(2):
            pp = mmps.tile([128, 512], f32, name="pp", tag="mm")
            for p in range(4):
                nc.tensor.matmul(
                    pp, yt[:, p, tb * 128:(tb + 1) * 128],
                    wp_sb[:, p, ns * 512:(ns + 1) * 512],
                    start=(p == 0), stop=(p == 3))
            nc.vector.tensor_copy(o_sb[:, ns * 512:(ns + 1) * 512], pp)
        if 'nostore' not in opts:
            if 'storesync' in opts:
                nc.sync.dma_start(out_d[tb * 128:(tb + 1) * 128, :], o_sb)
            else:
                nc.gpsimd.dma_start(out_d[tb * 128:(tb + 1) * 128, :], o_sb)

    # ---- segment 0 QKV: V first, then per-pr QK so attention(0, pr)
    # can start as soon as its own Q/K blocks land ----
    eager0 = 'eager0' in opts
    if 'B' in phases:
        for tb in range(4):
            emit_v(tb)
        if not eager0:
            if use_kd:
                for pr in range(4):
                    if krep:
                        emit_qk(pr, 0)
                        emit_kh(2 * pr, 0)
                        emit_kh(2 * pr + 1, 0)
                    else:
                        emit_qk(2 * pr, 0)
                        emit_qk(2 * pr + 1, 0)
                    emit_repl(0, pr)
            else:
                for mb in range(8):
                    emit_qk(mb, 0)

    # ---- steady state: attention(ts) with next-segment QKV and
    # previous-segment proj pieces interleaved to fill PE gaps ----
    projend = 'projil' not in opts
    for ts in range(NSEG):
        for pr in range(4):
            if ts == 0 and eager0 and 'B' in phases:
                emit_qk(2 * pr, 0)
                emit_qk(2 * pr + 1, 0)
                if use_kd:
                    emit_repl(0, pr)
            if 'C' in phases:
                emit_attention(ts, pr)
            if ts < NSEG - 1 and 'B' in phases:
                emit_v(4 * (ts + 1) + pr)
                if krep:
                    emit_qk(pr, (ts + 1) * 512)
                    emit_kh(2 * pr, (ts + 1) * 512)
                    emit_kh(2 * pr + 1, (ts + 1) * 512)
                else:
                    emit_qk(2 * pr, (ts + 1) * 512)
                    emit_qk(2 * pr + 1, (ts + 1) * 512)
                if use_kd:
                    emit_repl(ts + 1, pr)
            if not projend and ts > 0 and 'D' in phases:
                emit_proj(4 * (ts - 1) + pr)
        if projend and 'D' in phases:
            for tb in range(4 * ts, 4 * ts + 4):
                emit_proj(tb)
    if not projend and 'D' in phases:
        for tb in range(4 * (NSEG - 1), 4 * NSEG):
            emit_proj(tb)


def _shard_inputs(x, w_attn, b_attn, w_proj, krep=None):
    """Build per-core input maps (pair-packed q/k layouts; see module doc).

    krep=True packs wqkv as [q-pair0..3 | k-head0..7 with duplicated
    columns | v] (2048 cols); None resolves from the cached build opts.
    """
    if krep is None:
        krep = 'krep' in _cache.get("opts", ())
    wq = w_attn[:, 0:C].reshape(C, N_HEAD, D)
    wk = w_attn[:, C:2 * C].reshape(C, N_HEAD, D)
    wv = w_attn[:, 2 * C:3 * C].reshape(C, N_HEAD, D)
    bq = b_attn[0:C].reshape(N_HEAD, D)
    bk = b_attn[C:2 * C].reshape(N_HEAD, D)
    bv = b_attn[2 * C:3 * C].reshape(N_HEAD, D)

    xt_by_batch = [
        np.ascontiguousarray(x[b].T).astype(ml_dtypes.bfloat16)
        for b in range(B)
    ]

    in_maps = []
    for core in range(N_CORES):
        b, g = core // 2, core % 2
        h0 = g * H_LOC
        qk_blocks, bqk_parts = [], []
        if krep:
            for p in range(4):
                hA, hB = h0 + 2 * p, h0 + 2 * p + 1
                qk_blocks.append(
                    np.concatenate([wq[:, hA], wq[:, hB]], axis=1))
                bqk_parts.append(np.concatenate([bq[hA], bq[hB]]))
                bqk_parts.append(np.zeros(128, np.float32))
            for h in range(h0, h0 + H_LOC):
                qk_blocks.append(
                    np.concatenate([wk[:, h], wk[:, h]], axis=1))
        else:
            for p in range(4):
                hA, hB = h0 + 2 * p, h0 + 2 * p + 1
                qk_blocks.append(
                    np.concatenate([wq[:, hA], wq[:, hB]], axis=1))
                qk_blocks.append(
                    np.concatenate([wk[:, hA], wk[:, hB]], axis=1))
                bqk_parts.append(np.concatenate([bq[hA], bq[hB]]))
                bqk_parts.append(np.concatenate([bk[hA], bk[hB]]))
        wqkv = np.concatenate(
            qk_blocks + [wv[:, h0:h0 + H_LOC].reshape(C, H_LOC * D)], axis=1)
        bqkv = np.concatenate(
            bqk_parts + [bv[h0:h0 + H_LOC].reshape(H_LOC * D)])
        wproj = w_proj.reshape(N_HEAD, D, C)[h0:h0 + H_LOC].reshape(
            H_LOC * D, C)
        in_maps.append({
            "xt": xt_by_batch[b],
            "wqkv": np.ascontiguousarray(wqkv).astype(ml_dtypes.bfloat16),
            "bqkv": np.ascontiguousarray(bqkv, dtype=np.float32),
            "wproj": np.ascontiguousarray(wproj).astype(ml_dtypes.bfloat16),
        })
    return in_maps


def kernel(x, w_attn, b_attn, w_proj, b_proj):
    global last_exec_ns
    from concourse.bass_utils import run_bass_kernel_spmd

    x = np.asarray(x, dtype=np.float32)
    w_attn = np.asarray(w_attn, dtype=np.float32)
    b_attn = np.asarray(b_attn, dtype=np.float32)
    w_proj = np.asarray(w_proj, dtype=np.float32)
    b_proj = np.asarray(b_proj, dtype=np.float32)

    kenv = os.environ.get("KOPTS")
    if kenv is not None:
        kopts = tuple(o for o in kenv.split(",") if o)
    else:
        # default: K=128 block-diagonal attention; when the qkv bias is
        # zero (it is for this problem), also project K per-head with
        # duplicated stationary columns so kdiag fills via lane-aligned
        # DVE copies instead of DMAs
        if not np.any(b_attn):
            kopts = ("kdiag", "krep", "nobias")
        else:
            kopts = ("kdiag",)
    if _cache.get("opts") != kopts:
        _cache["nc"] = _build_program(opts=kopts)
        _cache["opts"] = kopts
    nc = _cache["nc"]

    in_maps = _shard_inputs(x, w_attn, b_attn, w_proj,
                            krep='krep' in kopts)
    trace = os.environ.get("KERNEL_TRACE", "0") == "1"
    if trace:
        try:
            import antenv.axon_hooks  # noqa: F401
        except ImportError:
            trace = False
    res = run_bass_kernel_spmd(nc, in_maps, core_ids=list(range(N_CORES)),
                               trace=trace)
    last_exec_ns = res.exec_time_ns

    out = np.empty((B, T, C), dtype=np.float32)
    for b in range(B):
        out[b] = (res.results[2 * b]["out"].astype(np.float32)
                  + res.results[2 * b + 1]["out"].astype(np.float32)
                  + b_proj[None, :])
    return out

